# revision 1
# baseline (speedup 1.0000x reference)
"""Multi-head attention with q/v LoRA on 8 trn2 NeuronCores.

Reference computation (B=2, N=2048, C=1024, H=16, HD=64, R=16):
    qkv = x @ w_qkv + b_qkv                -> split per-head q, k, v
    q  += ((q @ a_q) @ b_q) * 2.0          (per head; same for v)
    out = softmax(q k^T / 8) v             (full N x N scores)
    y   = out @ w_proj + b_proj

Sharding: tensor-parallel over heads for qkv+attention -- each of the 8
cores owns 2 heads (128 of the 1024 qkv columns) for both batches; the
attention output is then resharded over tokens with a 2 MB AllToAll so
each core computes final (not partial) proj rows for its 256 tokens per
batch with the full w_proj.  Per core:
  1. load host-pretransposed x^T chunks, round to fp32r, compute the
     qkv^T shard (weights stationary, x^T moving),
  2. LoRA via block-diagonal [128,32]/[32,128] matrices,
  3. per (batch, head, q-half): scores S^T = k^T' q^T -> exp on ACT ->
     P @ [v | 1] accumulated in PSUM (ones column yields softmax sums),
     normalize with a PE ones-broadcast of the reciprocal sums, DMA the
     normalized O^T slices into the per-batch AllToAll staging buffer,
  4. AllToAll [8, 128, 256] per batch, then proj y^T[:, my 256 tokens]
     = sum_k w_proj[k-chunk]^T @ recv[k-chunk] with bias on every core.
Emission order interleaves batch 1's qkv phase between batch 0's
attention and proj so the (static per-engine) schedule keeps the PE busy
while batch 0's AllToAll is in flight.
The host stitches the 8 token shards and transposes back to [B, N, C].
"""

import sys

sys.path.insert(0, "/opt/trn_rl_repo")
sys.path.insert(0, "/root/.axon_site")

import numpy as np

import concourse.bass as bass
import concourse.mybir as mybir
import concourse.tile as tile
from concourse.bass_utils import run_bass_kernel_spmd

f32 = mybir.dt.float32
f32r = mybir.dt.float32r
AF = mybir.ActivationFunctionType

B, N, C = 2, 2048, 1024
H, HD, R = 16, 64, 16
LORA_SCALE = 32.0 / R
ATTN_SCALE = HD ** -0.5
NCORES = 8
HPC = H // NCORES          # heads per core = 2
PC = HPC * HD              # partition columns per core = 128
ROWS = B * N               # 4096 tokens
RC = 256                   # row-chunk size for qkv production
TPC = N // NCORES          # tokens per core per batch = 256


def _legalize_waits(nc, max_waits=1):
    """This walrus build accepts at most one sync-wait per instruction;
    Tile attaches several.  Move surplus waits onto same-engine NoOps
    inserted immediately before the instruction (identical semantics)."""
    counter = 0
    for fn in nc.m.functions:
        for bb in fn.blocks:
            insts = bb.instructions
            out = []
            changed = False
            for inst in insts:
                si = inst.sync_info
                if si is not None and si.on_wait and len(si.on_wait) > max_waits:
                    waits = list(si.on_wait)
                    for w in waits[:-max_waits]:
                        counter += 1
                        nop = mybir.InstNoOp(
                            name=f"I-wfix-{counter}",
                            engine=inst.engine,
                            sync_info=mybir.SyncInfo(on_wait=[w], on_update=[]),
                        )
                        nc.register_instruction(nop)
                        out.append(nop)
                    si.on_wait.clear()
                    si.on_wait.extend(waits[-max_waits:])
                    changed = True
                out.append(inst)
            if changed:
                insts[:] = out


def build_nc():
    nc = bass.Bass(num_devices=NCORES)

    xt_d = nc.dram_tensor("xt", [C, ROWS], f32, kind="ExternalInput")
    wq_d = nc.dram_tensor("wq", [128, 1024], f32, kind="ExternalInput")
    wk_d = nc.dram_tensor("wk", [128, 1024], f32, kind="ExternalInput")
    wv_d = nc.dram_tensor("wv", [128, 1024], f32, kind="ExternalInput")
    bq_d = nc.dram_tensor("bq", [128, 1], f32, kind="ExternalInput")
    bk_d = nc.dram_tensor("bk", [128, 1], f32, kind="ExternalInput")
    bv_d = nc.dram_tensor("bv", [128, 1], f32, kind="ExternalInput")
    a2q_d = nc.dram_tensor("a2q", [128, 2 * R], f32, kind="ExternalInput")
    b2q_d = nc.dram_tensor("b2q", [2 * R, 128], f32, kind="ExternalInput")
    a2v_d = nc.dram_tensor("a2v", [128, 2 * R], f32, kind="ExternalInput")
    b2v_d = nc.dram_tensor("b2v", [2 * R, 128], f32, kind="ExternalInput")
    wp_d = nc.dram_tensor("wp", [128, 8 * 1024], f32, kind="ExternalInput")
    bp_d = nc.dram_tensor("bp", [128, 8], f32, kind="ExternalInput")
    eye64x2_d = nc.dram_tensor("eye64x2", [128, 64], f32, kind="ExternalInput")
    out_d = nc.dram_tensor("out", [B, C, TPC], f32, kind="ExternalOutput")

    with nc.allow_low_precision(
        reason="fp32r rounding is intended; PSUM accumulation stays fp32"
    ), tile.TileContext(nc) as tc:
        with (
            tc.tile_pool(name="persist", bufs=1) as persist,
            tc.tile_pool(name="const", bufs=1) as const,
            tc.tile_pool(name="dram", bufs=1, space="DRAM") as dram,
            tc.tile_pool(name="xio", bufs=2) as xio_p,
            tc.tile_pool(name="work", bufs=2) as work_p,
            tc.tile_pool(name="ps", bufs=1, space="PSUM") as ps,
        ):
            qT = persist.tile([128, ROWS], f32r, tag="qT", name="qT")
            kT = persist.tile([128, ROWS], f32r, tag="kT", name="kT")
            vT = persist.tile([128, ROWS], f32r, tag="vT", name="vT")

            # prefetch the first x^T chunk's DMA ahead of the weight DMAs
            xstg00 = xio_p.tile([128, 8 * RC], f32, tag="xstg", name="xs00")
            nc.sync.dma_start(
                out=xstg00[:].rearrange("p (a r) -> p a r", a=8),
                in_=xt_d[:, 0:RC].rearrange("(a p) r -> p a r", p=128),
            )

            # fp32 staging + on-device rounding to fp32r for matmul operands
            def rounded(name, dram_t, shape, eng="v"):
                stg = const.tile(list(shape), f32, tag="stg", name=f"{name}_stg")
                nc.sync.dma_start(out=stg[:], in_=dram_t[:])
                t = const.tile(list(shape), f32r, tag=name, name=name)
                if eng == "v":
                    nc.vector.tensor_copy(t[:], stg[:])
                else:
                    nc.scalar.activation(t[:], stg[:], AF.Copy)
                return t

            w_t = [
                rounded("wq_t", wq_d, (128, 1024)),
                rounded("wk_t", wk_d, (128, 1024), eng="s"),
                rounded("wv_t", wv_d, (128, 1024), eng="s"),
            ]
            a2q_t = rounded("a2q_t", a2q_d, (128, 2 * R), eng="s")
            b2q_t = rounded("b2q_t", b2q_d, (2 * R, 128), eng="s")
            a2v_t = rounded("a2v_t", a2v_d, (128, 2 * R), eng="s")
            b2v_t = rounded("b2v_t", b2v_d, (2 * R, 128), eng="s")

            eye64x2_s = const.tile([128, 64], f32, tag="eye64s", name="eye64s")
            nc.sync.dma_start(out=eye64x2_s[:], in_=eye64x2_d[:])
            eye64x2 = const.tile([128, 64], f32r, tag="eye64", name="eye64")
            nc.vector.tensor_copy(eye64x2[:], eye64x2_s[:])

            ones_s = const.tile([128, 64], f32, tag="ones_s", name="ones_s")
            nc.gpsimd.memset(ones_s[:], 1.0)
            ones_row = const.tile([1, 64], f32r, tag="ones_r", name="ones_r")
            nc.vector.tensor_copy(ones_row[:], ones_s[0:1, :])
            ones_col = const.tile([128, 1], f32r, tag="ones_c", name="ones_c")
            nc.vector.tensor_copy(ones_col[:], ones_s[:, 0:1])

            bias_t = []
            for nm, d in (("bq", bq_d), ("bk", bk_d), ("bv", bv_d)):
                bt = const.tile([128, 1], f32, tag=nm, name=f"{nm}_t")
                nc.sync.dma_start(out=bt[:], in_=d[:])
                bias_t.append(bt)
            bp_t = const.tile([128, 8], f32, tag="bp", name="bp_t")
            nc.sync.dma_start(out=bp_t[:], in_=bp_d[:])

            wp_t = const.tile([128, 8 * 1024], f32r, tag="wp_t", name="wp_t")

            qkvT = (qT, kT, vT)

            def load_xchunk(b, rci, xstg=None, act_ok=True):
                r0 = b * N + rci * RC
                if xstg is None:
                    xstg = xio_p.tile([128, 8 * RC], f32, tag="xstg", name=f"xs{b}{rci}")
                    nc.sync.dma_start(
                        out=xstg[:].rearrange("p (a r) -> p a r", a=8),
                        in_=xt_d[:, r0 : r0 + RC].rearrange("(a p) r -> p a r", p=128),
                    )
                xT_t = xio_p.tile([128, 8 * RC], f32r, tag="xT", name=f"xT{b}{rci}")
                for ci in range(8):
                    sl = slice(ci * RC, (ci + 1) * RC)
                    if act_ok and ci % 2 == 1:
                        nc.scalar.activation(xT_t[:, sl], xstg[:, sl], AF.Copy)
                    else:
                        nc.vector.tensor_copy(xT_t[:, sl], xstg[:, sl])
                return xT_t

            def emit_qkv_chunk(b, rci, xT_t=None, act_ok=True):
                r0 = b * N + rci * RC
                if xT_t is None:
                    xT_t = load_xchunk(b, rci, act_ok=act_ok)
                for m in range(3):
                    acc = ps.tile([128, RC], f32, tag="acc", bufs=2, name=f"ac{b}{rci}{m}")
                    for ci in range(8):
                        nc.tensor.matmul(
                            acc[:],
                            w_t[m][:, ci * 128 : (ci + 1) * 128],
                            xT_t[:, ci * RC : (ci + 1) * RC],
                            start=(ci == 0),
                            stop=(ci == 7),
                        )
                    dst = qkvT[m][:, r0 : r0 + RC]
                    if m == 0 and act_ok:
                        nc.scalar.activation(dst, acc[:], AF.Identity, bias=bias_t[m][:])
                    else:
                        nc.vector.tensor_scalar_add(dst, acc[:], bias_t[m][:])

            def emit_lora(b, ch):
                boff = b * N
                for dstT, a2, b2 in ((qT, a2q_t, b2q_t), (vT, a2v_t, b2v_t)):
                    sl = slice(boff + ch * 512, boff + (ch + 1) * 512)
                    t_ps = ps.tile([2 * R, 512], f32, tag="s", bufs=2, name=f"tl{b}{ch}")
                    nc.tensor.matmul(t_ps[:], a2[:], dstT[:, sl], start=True, stop=True)
                    t_sb = work_p.tile([2 * R, 512], f32r, tag="lt", name=f"ts{b}{ch}")
                    nc.vector.tensor_copy(t_sb[:], t_ps[:])
                    d_ps = ps.tile([128, 512], f32, tag="s", bufs=2, name=f"dl{b}{ch}")
                    nc.tensor.matmul(d_ps[:], b2[:], t_sb[:], start=True, stop=True)
                    nc.vector.tensor_add(dstT[:, sl], dstT[:, sl], d_ps[:])

            def emit_vaug(b, hl):
                boff = b * N
                hs = slice(hl * HD, (hl + 1) * HD)
                v_aug = work_p.tile([128, 16 * 65], f32r, tag="vaug", name=f"va{b}{hl}")
                for kt in range(16):
                    ko = boff + kt * 128
                    vtr = ps.tile([128, 64], f32r, tag="s", bufs=2, name=f"vt{b}{hl}{kt}")
                    nc.tensor.transpose(vtr[:], vT[hs, ko : ko + 128], eye64x2[hs, :])
                    nc.vector.tensor_copy(v_aug[:, kt * 65 : kt * 65 + 64], vtr[:])
                    nc.vector.tensor_copy(
                        v_aug[:, kt * 65 + 64 : kt * 65 + 65], ones_col[:]
                    )
                return v_aug

            def emit_unit(b, hl, qh, v_aug, a2a_in):
                boff = b * N
                hs = slice(hl * HD, (hl + 1) * HD)
                qoff = boff + qh * 1024
                o_ps = ps.tile([65, 1024], f32, tag="o", bufs=1, name=f"o{b}{hl}{qh}")

                def emit_pv(p_tile, kt):
                    for qc in range(2):
                        nc.tensor.matmul(
                            o_ps[:, qc * 512 : (qc + 1) * 512],
                            v_aug[:, kt * 65 : kt * 65 + 65],
                            p_tile[:, qc * 512 : (qc + 1) * 512],
                            start=(kt == 0),
                            stop=(kt == 15),
                        )

                pending = None
                for kt in range(16):
                    ko = boff + kt * 128
                    s_ps = ps.tile([128, 1024], f32, tag="s", bufs=2, name=f"s{b}{hl}{qh}{kt}")
                    for qc in range(2):
                        nc.tensor.matmul(
                            s_ps[:, qc * 512 : (qc + 1) * 512],
                            kT[hs, ko : ko + 128],
                            qT[hs, qoff + qc * 512 : qoff + (qc + 1) * 512],
                            start=True,
                            stop=True,
                        )
                    p_sb = work_p.tile([128, 1024], f32r, tag="p", bufs=3, name=f"p{qh}{kt}")
                    nc.scalar.activation(p_sb[:], s_ps[:], AF.Exp, scale=ATTN_SCALE)
                    if pending is not None:
                        emit_pv(*pending)
                    pending = (p_sb, kt)
                emit_pv(*pending)
                # copy O^T+sums out of PSUM first (releases o fast), then
                # normalize off the critical path
                nst = work_p.tile([65, 1024], f32, tag="nst", bufs=2, name=f"n{hl}{qh}")
                nc.vector.tensor_copy(nst[:], o_ps[:])
                r_sb = work_p.tile([1, 1024], f32r, tag="r", bufs=2, name=f"r{b}{hl}{qh}")
                nc.vector.reciprocal(r_sb[:], nst[64:65, :])
                for qc in range(2):
                    bc_ps = ps.tile([64, 512], f32, tag="acc", bufs=2, name=f"bc{qc}")
                    nc.tensor.matmul(
                        bc_ps[:],
                        ones_row[:],
                        r_sb[:, qc * 512 : (qc + 1) * 512],
                        start=True,
                        stop=True,
                    )
                    bcs = work_p.tile([64, 512], f32, tag="bcs", bufs=2, name=f"bs{qc}")
                    nc.vector.tensor_copy(bcs[:], bc_ps[:])
                    nc.vector.tensor_mul(
                        nst[0:64, qc * 512 : (qc + 1) * 512],
                        nst[0:64, qc * 512 : (qc + 1) * 512],
                        bcs[:],
                    )
                for tci in range(4):
                    nc.sync.dma_start(
                        out=a2a_in[qh * 4 + tci, :, :],
                        in_=nst[0:64, tci * TPC : (tci + 1) * TPC],
                    )

            def emit_a2a(b, hl, a2a_in):
                a2a_out = dram.tile(
                    [8, 64, TPC], f32, tag=f"ao{b}{hl}", name=f"ao{b}{hl}"
                )
                nc.gpsimd.collective_compute(
                    "AllToAll",
                    mybir.AluOpType.bypass,
                    replica_groups=[list(range(NCORES))],
                    ins=[a2a_in[:].opt()],
                    outs=[a2a_out[:].opt()],
                )
                return a2a_out

            def new_a2a_in(b, hl):
                return dram.tile([8, 64, TPC], f32, tag=f"ai{b}{hl}", name=f"ai{b}{hl}")

            recv_tiles = {}

            def get_recv(b):
                if b not in recv_tiles:
                    recv_tiles[b] = work_p.tile(
                        [128, 8 * TPC], f32r, tag=f"rcr{b}", bufs=1, name=f"rr{b}"
                    )
                return recv_tiles[b]

            def emit_recv_head(b, hl, a2a_out):
                recv_r = get_recv(b)
                for kc in range(8):
                    rstg = work_p.tile([64, TPC], f32, tag="rst", bufs=3, name=f"rg{b}{hl}{kc}")
                    nc.sync.dma_start(out=rstg[:], in_=a2a_out[kc])
                    nc.vector.tensor_copy(
                        recv_r[hl * 64 : (hl + 1) * 64, kc * TPC : (kc + 1) * TPC],
                        rstg[:],
                    )
                return recv_r

            def emit_proj_mt(b, recv_r, mts):
                for mt in mts:
                    y_ps = ps.tile([128, TPC], f32, tag="acc", bufs=2, name=f"y{b}{mt}")
                    for kc in range(8):
                        nc.tensor.matmul(
                            y_ps[:],
                            wp_t[:, kc * 1024 + mt * 128 : kc * 1024 + (mt + 1) * 128],
                            recv_r[:, kc * TPC : (kc + 1) * TPC],
                            start=(kc == 0),
                            stop=(kc == 7),
                        )
                    yst = work_p.tile([128, TPC], f32, tag="yst", bufs=3, name=f"ys{b}{mt}")
                    nc.vector.tensor_scalar_add(yst[:], y_ps[:], bp_t[:, mt : mt + 1])
                    nc.sync.dma_start(
                        out=out_d[b, mt * 128 : (mt + 1) * 128, :], in_=yst[:]
                    )

            # ---- interleaved emission schedule ----------------------------
            emit_qkv_chunk(0, 0, xT_t=load_xchunk(0, 0, xstg=xstg00))
            for rci in range(1, 8):
                emit_qkv_chunk(0, rci)
            for ch in range(4):
                emit_lora(0, ch)

            ai = {(0, 0): new_a2a_in(0, 0), (0, 1): new_a2a_in(0, 1)}
            ao = {}
            va = emit_vaug(0, 0)
            emit_unit(0, 0, 0, va, ai[(0, 0)])
            emit_qkv_chunk(1, 0, act_ok=False)
            emit_qkv_chunk(1, 1, act_ok=False)
            emit_unit(0, 0, 1, va, ai[(0, 0)])
            ao[(0, 0)] = emit_a2a(0, 0, ai[(0, 0)])
            emit_qkv_chunk(1, 2, act_ok=False)
            emit_qkv_chunk(1, 3, act_ok=False)
            emit_lora(1, 0)
            va = emit_vaug(0, 1)
            emit_unit(0, 1, 0, va, ai[(0, 1)])
            emit_qkv_chunk(1, 4, act_ok=False)
            emit_qkv_chunk(1, 5, act_ok=False)
            emit_lora(1, 1)
            emit_unit(0, 1, 1, va, ai[(0, 1)])
            ao[(0, 1)] = emit_a2a(0, 1, ai[(0, 1)])
            emit_qkv_chunk(1, 6, act_ok=False)
            emit_qkv_chunk(1, 7, act_ok=False)
            emit_lora(1, 2)
            emit_lora(1, 3)
            # round full w_proj (first needed by proj(0))
            for kc in range(8):
                stg = const.tile([128, 1024], f32, tag="stg", name=f"wp_stg{kc}")
                nc.sync.dma_start(out=stg[:], in_=wp_d[:, kc * 1024 : (kc + 1) * 1024])
                if kc % 2 == 0:
                    nc.vector.tensor_copy(wp_t[:, kc * 1024 : (kc + 1) * 1024], stg[:])
                else:
                    nc.scalar.activation(
                        wp_t[:, kc * 1024 : (kc + 1) * 1024], stg[:], AF.Copy
                    )
            emit_recv_head(0, 0, ao[(0, 0)])
            recv0 = emit_recv_head(0, 1, ao[(0, 1)])

            ai = {(1, 0): new_a2a_in(1, 0), (1, 1): new_a2a_in(1, 1)}
            va = emit_vaug(1, 0)
            emit_unit(1, 0, 0, va, ai[(1, 0)])
            emit_proj_mt(0, recv0, range(0, 4))
            emit_unit(1, 0, 1, va, ai[(1, 0)])
            ao[(1, 0)] = emit_a2a(1, 0, ai[(1, 0)])
            emit_proj_mt(0, recv0, range(4, 8))
            emit_recv_head(1, 0, ao[(1, 0)])
            va = emit_vaug(1, 1)
            emit_unit(1, 1, 0, va, ai[(1, 1)])
            emit_unit(1, 1, 1, va, ai[(1, 1)])
            ao[(1, 1)] = emit_a2a(1, 1, ai[(1, 1)])
            recv1 = emit_recv_head(1, 1, ao[(1, 1)])
            emit_proj_mt(1, recv1, range(8))

    _legalize_waits(nc)
    return nc


_NC_CACHE = None


def _get_nc():
    global _NC_CACHE
    if _NC_CACHE is None:
        _NC_CACHE = build_nc()
    return _NC_CACHE


def _make_in_maps(inputs):
    x = np.ascontiguousarray(np.asarray(inputs["x"], dtype=np.float32)).reshape(ROWS, C)
    xt = np.ascontiguousarray(x.T)            # [C, ROWS]
    w_qkv = np.asarray(inputs["w_qkv"], dtype=np.float32)
    b_qkv = np.asarray(inputs["b_qkv"], dtype=np.float32)
    a_q = np.asarray(inputs["a_q"], dtype=np.float32)
    b_q = np.asarray(inputs["b_q"], dtype=np.float32)
    a_v = np.asarray(inputs["a_v"], dtype=np.float32)
    b_v = np.asarray(inputs["b_v"], dtype=np.float32)
    w_proj = np.asarray(inputs["w_proj"], dtype=np.float32)
    b_proj = np.asarray(inputs["b_proj"], dtype=np.float32)

    def blkdiag(m):
        z = np.zeros((2 * m.shape[0], 2 * m.shape[1]), dtype=np.float32)
        z[: m.shape[0], : m.shape[1]] = m
        z[m.shape[0] :, m.shape[1] :] = m
        return z

    a2q = blkdiag(a_q)
    b2q = blkdiag(b_q) * LORA_SCALE
    a2v = blkdiag(a_v)
    b2v = blkdiag(b_v) * LORA_SCALE
    eye64x2 = np.vstack([np.eye(64, dtype=np.float32)] * 2)

    def warr(w):                              # [1024, n] -> [128, 8*n] chunk-major
        n = w.shape[1]
        return np.ascontiguousarray(
            w.reshape(8, 128, n).transpose(1, 0, 2).reshape(128, 8 * n)
        )

    wp_full = warr(w_proj)                    # [128, 8*1024]
    bp = np.ascontiguousarray(b_proj.reshape(8, 128).T)

    in_maps = []
    for c in range(NCORES):
        in_maps.append(
            {
                "xt": xt,
                "wq": warr(w_qkv[:, 0 * C + c * PC : 0 * C + (c + 1) * PC]),
                "wk": warr(w_qkv[:, 1 * C + c * PC : 1 * C + (c + 1) * PC]),
                "wv": warr(w_qkv[:, 2 * C + c * PC : 2 * C + (c + 1) * PC]),
                "bq": np.ascontiguousarray(b_qkv[0 * C + c * PC : 0 * C + (c + 1) * PC].reshape(128, 1)),
                "bk": np.ascontiguousarray(b_qkv[1 * C + c * PC : 1 * C + (c + 1) * PC].reshape(128, 1)),
                "bv": np.ascontiguousarray(b_qkv[2 * C + c * PC : 2 * C + (c + 1) * PC].reshape(128, 1)),
                "a2q": a2q,
                "b2q": b2q,
                "a2v": a2v,
                "b2v": b2v,
                "wp": wp_full,
                "bp": bp,
                "eye64x2": eye64x2,
            }
        )
    return in_maps


def run_sharded(inputs, trace=False, **kw):
    nc = _get_nc()
    in_maps = _make_in_maps(inputs)
    res = run_bass_kernel_spmd(nc, in_maps, list(range(NCORES)), trace=trace, **kw)
    # results[c]["out"]: [B, C, TPC] -- core c's token shard of final y^T
    yT = np.concatenate([res.results[c]["out"] for c in range(NCORES)], axis=2)
    out = np.ascontiguousarray(yT.transpose(0, 2, 1))  # [B, N, C]
    return out, res


def kernel(**inputs) -> np.ndarray:
    out, _ = run_sharded(inputs, trace=False)
    return out



# revision 7
# speedup vs baseline: 1.1823x; 1.1823x over previous
"""Multi-head attention with q/v LoRA on 8 trn2 NeuronCores (bf16 PE path).

Reference computation (B=2, N=2048, C=1024, H=16, HD=64, R=16):
    qkv = x @ w_qkv + b_qkv                -> split per-head q, k, v
    q  += ((q @ a_q) @ b_q) * 2.0          (per head; same for v)
    out = softmax(q k^T / 8) v             (full N x N scores)
    y   = out @ w_proj + b_proj

Sharding: tensor-parallel over heads -- each of the 8 cores owns 2 heads
(128 of the 3*1024 qkv columns) for both batches; attention output is
resharded over tokens with a per-(batch,head) AllToAll so each core
computes final proj rows for its 256 tokens per batch with the full
w_proj.

Key implementation choices:
  * All PE operands are bf16 (hardware runs fp32r at ~2 cycles/row; bf16
    at 1).  PSUM accumulation stays fp32, biases stay fp32.
  * The LoRA is linear in q/v, so it is folded into the qkv weights on
    the host: w_eff = w @ (I + blockdiag(a@b)*scale), same for bias.
    Nothing LoRA-related runs on device.
  * x^T, weights are pre-cast to bf16 on the host and DMA'd straight
    into matmul operand tiles (no on-device rounding copies).
  * Softmax: scores S^T = k^T' q^T -> exp on ACT (bf16 out) -> P @ [v|1]
    in PSUM (ones column gives the row sums).  1/sums via the fast DVE
    reciprocal, broadcast to 64 partitions on the idle GpSimd engine,
    one fused multiply writes normalized bf16 O^T for the AllToAll.
  * v_aug ones columns are memset once into two persistent buffers.
  * AllToAll payloads are bf16 (256 KB per (batch, head)); the receive
    side DMAs the collective output straight into the proj operand tile.
The host stitches the 8 token shards and transposes back to [B, N, C].
"""

import sys

sys.path.insert(0, "/opt/trn_rl_repo")
sys.path.insert(0, "/root/.axon_site")

import numpy as np
import ml_dtypes

import concourse.bass as bass
import concourse.mybir as mybir
import concourse.tile as tile
from concourse.bass_utils import run_bass_kernel_spmd

f32 = mybir.dt.float32
bf16 = mybir.dt.bfloat16
AF = mybir.ActivationFunctionType

B, N, C = 2, 2048, 1024
H, HD, R = 16, 64, 16
LORA_SCALE = 32.0 / R
ATTN_SCALE = HD ** -0.5
NCORES = 8
HPC = H // NCORES          # heads per core = 2
PC = HPC * HD              # partition columns per core = 128
ROWS = B * N               # 4096 tokens
RC = 512                   # row-chunk size for qkv production
TPC = N // NCORES          # tokens per core per batch = 256


def _legalize_waits(nc, max_waits=1):
    """This walrus build accepts at most one sync-wait per instruction;
    Tile attaches several.  Move surplus waits onto same-engine NoOps
    inserted immediately before the instruction (identical semantics)."""
    counter = 0
    for fn in nc.m.functions:
        for bb in fn.blocks:
            insts = bb.instructions
            out = []
            changed = False
            for inst in insts:
                si = inst.sync_info
                if si is not None and si.on_wait and len(si.on_wait) > max_waits:
                    waits = list(si.on_wait)
                    for w in waits[:-max_waits]:
                        counter += 1
                        nop = mybir.InstNoOp(
                            name=f"I-wfix-{counter}",
                            engine=inst.engine,
                            sync_info=mybir.SyncInfo(on_wait=[w], on_update=[]),
                        )
                        nc.register_instruction(nop)
                        out.append(nop)
                    si.on_wait.clear()
                    si.on_wait.extend(waits[-max_waits:])
                    changed = True
                out.append(inst)
            if changed:
                insts[:] = out


def build_nc():
    nc = bass.Bass(num_devices=NCORES)

    xt_d = nc.dram_tensor("xt", [C, ROWS], bf16, kind="ExternalInput")
    wq_d = nc.dram_tensor("wq", [128, 1024], bf16, kind="ExternalInput")
    wk_d = nc.dram_tensor("wk", [128, 1024], bf16, kind="ExternalInput")
    wv_d = nc.dram_tensor("wv", [128, 1024], bf16, kind="ExternalInput")
    bq_d = nc.dram_tensor("bq", [128, 1], f32, kind="ExternalInput")
    bk_d = nc.dram_tensor("bk", [128, 1], f32, kind="ExternalInput")
    bv_d = nc.dram_tensor("bv", [128, 1], f32, kind="ExternalInput")
    wp_d = nc.dram_tensor("wp", [128, 8 * 1024], bf16, kind="ExternalInput")
    bp_d = nc.dram_tensor("bp", [128, 8], f32, kind="ExternalInput")
    eye64x2_d = nc.dram_tensor("eye64x2", [128, 64], bf16, kind="ExternalInput")
    out_d = nc.dram_tensor("out", [B, C, TPC], f32, kind="ExternalOutput")

    with nc.allow_low_precision(
        reason="bf16 matmul operands are intended; PSUM accumulation stays fp32"
    ), tile.TileContext(nc) as tc:
        with (
            tc.tile_pool(name="persist", bufs=1) as persist,
            tc.tile_pool(name="const", bufs=1) as const,
            tc.tile_pool(name="dram", bufs=1, space="DRAM") as dram,
            tc.tile_pool(name="xio", bufs=2) as xio_p,
            tc.tile_pool(name="work", bufs=2) as work_p,
            tc.tile_pool(name="ps", bufs=1, space="PSUM") as ps,
        ):
            qT = persist.tile([128, ROWS], bf16, tag="qT", name="qT")
            kT = persist.tile([128, ROWS], bf16, tag="kT", name="kT")
            vT = persist.tile([128, ROWS], bf16, tag="vT", name="vT")

            # prefetch the first x^T chunk ahead of the weight DMAs
            xT0 = xio_p.tile([128, 8 * RC], bf16, tag="xT", name="xT00")
            nc.sync.dma_start(
                out=xT0[:].rearrange("p (a r) -> p a r", a=8),
                in_=xt_d[:, 0:RC].rearrange("(a p) r -> p a r", p=128),
            )

            w_t = []
            for nm, d in (("wq", wq_d), ("wk", wk_d), ("wv", wv_d)):
                t = const.tile([128, 1024], bf16, tag=nm, name=f"{nm}_t")
                nc.sync.dma_start(out=t[:], in_=d[:])
                w_t.append(t)

            eye64x2 = const.tile([128, 64], bf16, tag="eye64", name="eye64")
            nc.sync.dma_start(out=eye64x2[:], in_=eye64x2_d[:])

            bias_t = []
            for nm, d in (("bq", bq_d), ("bk", bk_d), ("bv", bv_d)):
                bt = const.tile([128, 1], f32, tag=nm, name=f"{nm}_t")
                nc.sync.dma_start(out=bt[:], in_=d[:])
                bias_t.append(bt)
            bp_t = const.tile([128, 8], f32, tag="bp", name="bp_t")
            nc.sync.dma_start(out=bp_t[:], in_=bp_d[:])

            wp_t = const.tile([128, 8 * 1024], bf16, tag="wp_t", name="wp_t")

            # persistent v_aug buffers: ones columns written once, data
            # blocks overwritten per (batch, head)
            v_aug_bufs = []
            for i in range(2):
                va = persist.tile([128, 16 * 65], bf16, tag=f"va{i}", name=f"va{i}")
                nc.gpsimd.memset(va[:], 1.0)
                v_aug_bufs.append(va)

            ones_row = const.tile([1, 64], bf16, tag="ones_r", name="ones_r")
            nc.gpsimd.memset(ones_row[:], 1.0)

            qkvT = (qT, kT, vT)

            def emit_qkv_chunk(b, rci, xT_t=None, act_ok=True):
                r0 = b * N + rci * RC
                if xT_t is None:
                    xT_t = xio_p.tile([128, 8 * RC], bf16, tag="xT", name=f"xT{b}{rci}")
                    nc.sync.dma_start(
                        out=xT_t[:].rearrange("p (a r) -> p a r", a=8),
                        in_=xt_d[:, r0 : r0 + RC].rearrange("(a p) r -> p a r", p=128),
                    )
                for m in range(3):
                    acc = ps.tile([128, RC], f32, tag="acc", bufs=2, name=f"ac{b}{rci}{m}")
                    for ci in range(8):
                        nc.tensor.matmul(
                            acc[:],
                            w_t[m][:, ci * 128 : (ci + 1) * 128],
                            xT_t[:, ci * RC : (ci + 1) * RC],
                            start=(ci == 0),
                            stop=(ci == 7),
                        )
                    dst = qkvT[m][:, r0 : r0 + RC]
                    if m == 0 and act_ok:
                        nc.scalar.activation(dst, acc[:], AF.Identity, bias=bias_t[m][:])
                    else:
                        nc.vector.tensor_scalar_add(dst, acc[:], bias_t[m][:])

            def emit_vaug(b, hl):
                boff = b * N
                hs = slice(hl * HD, (hl + 1) * HD)
                v_aug = v_aug_bufs[hl]
                for kt in range(16):
                    ko = boff + kt * 128
                    vtr = ps.tile([128, 64], bf16, tag="s", bufs=2, name=f"vt{b}{hl}{kt}")
                    nc.tensor.transpose(vtr[:], vT[hs, ko : ko + 128], eye64x2[hs, :])
                    nc.vector.tensor_copy(v_aug[:, kt * 65 : kt * 65 + 64], vtr[:])
                return v_aug

            def emit_unit(b, hl, qh, v_aug, a2a_in):
                boff = b * N
                hs = slice(hl * HD, (hl + 1) * HD)
                qoff = boff + qh * 1024
                o_ps = ps.tile([65, 1024], f32, tag="o", bufs=1, name=f"o{b}{hl}{qh}")

                def emit_pv(p_tile, kt):
                    for qc in range(2):
                        nc.tensor.matmul(
                            o_ps[:, qc * 512 : (qc + 1) * 512],
                            v_aug[:, kt * 65 : kt * 65 + 65],
                            p_tile[:, qc * 512 : (qc + 1) * 512],
                            start=(kt == 0),
                            stop=(kt == 15),
                        )

                pending = None
                for kt in range(16):
                    ko = boff + kt * 128
                    s_ps = ps.tile([128, 1024], f32, tag="s", bufs=2, name=f"s{b}{hl}{qh}{kt}")
                    for qc in range(2):
                        nc.tensor.matmul(
                            s_ps[:, qc * 512 : (qc + 1) * 512],
                            kT[hs, ko : ko + 128],
                            qT[hs, qoff + qc * 512 : qoff + (qc + 1) * 512],
                            start=True,
                            stop=True,
                        )
                    p_sb = work_p.tile([128, 1024], bf16, tag="p", bufs=3, name=f"p{qh}{kt}")
                    nc.scalar.activation(p_sb[:], s_ps[:], AF.Exp, scale=ATTN_SCALE)
                    if pending is not None:
                        emit_pv(*pending)
                    pending = (p_sb, kt)
                emit_pv(*pending)
                # pull O^T + sums out of PSUM (releases the o bank), then
                # normalize: 1/sums on DVE, partition-broadcast on GpSimd,
                # one multiply writes the bf16 AllToAll payload
                nst = work_p.tile([65, 1024], f32, tag="nst", bufs=2, name=f"n{hl}{qh}")
                nc.vector.tensor_copy(nst[:], o_ps[:])
                r_sb = work_p.tile([1, 1024], f32, tag="r", bufs=2, name=f"r{b}{hl}{qh}")
                nc.vector.reciprocal(r_sb[:], nst[64:65, :])
                r_bf = work_p.tile([1, 1024], bf16, tag="rbf", bufs=2, name=f"rb{b}{hl}{qh}")
                nc.vector.tensor_copy(r_bf[:], r_sb[:])
                onrm = work_p.tile([64, 1024], bf16, tag="onrm", bufs=2, name=f"on{hl}{qh}")
                for qc in range(2):
                    bc_ps = ps.tile([64, 512], f32, tag="acc", bufs=2, name=f"bc{qc}")
                    nc.tensor.matmul(
                        bc_ps[:],
                        ones_row[:],
                        r_bf[:, qc * 512 : (qc + 1) * 512],
                        start=True,
                        stop=True,
                    )
                    nc.vector.tensor_mul(
                        onrm[:, qc * 512 : (qc + 1) * 512],
                        nst[0:64, qc * 512 : (qc + 1) * 512],
                        bc_ps[:],
                    )
                nc.sync.dma_start(
                    out=a2a_in[qh * 4 : (qh + 1) * 4, :, :].rearrange("a p r -> p a r"),
                    in_=onrm[:].rearrange("p (a r) -> p a r", a=4),
                )

            def emit_a2a(b, hl, a2a_in):
                a2a_out = dram.tile(
                    [8, 64, TPC], bf16, tag=f"ao{b}{hl}", name=f"ao{b}{hl}"
                )
                nc.gpsimd.collective_compute(
                    "AllToAll",
                    mybir.AluOpType.bypass,
                    replica_groups=[list(range(NCORES))],
                    ins=[a2a_in[:].opt()],
                    outs=[a2a_out[:].opt()],
                )
                return a2a_out

            def new_a2a_in(b, hl):
                return dram.tile([8, 64, TPC], bf16, tag=f"ai{b}{hl}", name=f"ai{b}{hl}")

            recv_tiles = {}

            def get_recv(b):
                if b not in recv_tiles:
                    recv_tiles[b] = work_p.tile(
                        [128, 8 * TPC], bf16, tag=f"rcr{b}", bufs=1, name=f"rr{b}"
                    )
                return recv_tiles[b]

            def emit_recv_head(b, hl, a2a_out):
                recv_r = get_recv(b)
                nc.sync.dma_start(
                    out=recv_r[hl * 64 : (hl + 1) * 64, :].rearrange(
                        "p (a r) -> p a r", a=8
                    ),
                    in_=a2a_out[:].rearrange("a p r -> p a r"),
                )
                return recv_r

            def emit_proj_mt(b, recv_r, mts):
                for mt in mts:
                    y_ps = ps.tile([128, TPC], f32, tag="acc", bufs=2, name=f"y{b}{mt}")
                    for kc in range(8):
                        nc.tensor.matmul(
                            y_ps[:],
                            wp_t[:, kc * 1024 + mt * 128 : kc * 1024 + (mt + 1) * 128],
                            recv_r[:, kc * TPC : (kc + 1) * TPC],
                            start=(kc == 0),
                            stop=(kc == 7),
                        )
                    yst = work_p.tile([128, TPC], f32, tag="yst", bufs=3, name=f"ys{b}{mt}")
                    nc.vector.tensor_scalar_add(yst[:], y_ps[:], bp_t[:, mt : mt + 1])
                    nc.sync.dma_start(
                        out=out_d[b, mt * 128 : (mt + 1) * 128, :], in_=yst[:]
                    )

            # ---- interleaved emission schedule ----------------------------
            emit_qkv_chunk(0, 0, xT_t=xT0)
            for rci in range(1, 4):
                emit_qkv_chunk(0, rci)

            ai = {(0, 0): new_a2a_in(0, 0), (0, 1): new_a2a_in(0, 1)}
            ao = {}
            va = emit_vaug(0, 0)
            emit_unit(0, 0, 0, va, ai[(0, 0)])
            emit_qkv_chunk(1, 0, act_ok=False)
            emit_qkv_chunk(1, 1, act_ok=False)
            emit_unit(0, 0, 1, va, ai[(0, 0)])
            ao[(0, 0)] = emit_a2a(0, 0, ai[(0, 0)])
            emit_qkv_chunk(1, 2, act_ok=False)
            emit_qkv_chunk(1, 3, act_ok=False)
            va = emit_vaug(0, 1)
            emit_unit(0, 1, 0, va, ai[(0, 1)])
            emit_unit(0, 1, 1, va, ai[(0, 1)])
            ao[(0, 1)] = emit_a2a(0, 1, ai[(0, 1)])
            # full w_proj DMA (first needed by proj(0))
            nc.sync.dma_start(out=wp_t[:], in_=wp_d[:])
            emit_recv_head(0, 0, ao[(0, 0)])
            recv0 = emit_recv_head(0, 1, ao[(0, 1)])

            ai = {(1, 0): new_a2a_in(1, 0), (1, 1): new_a2a_in(1, 1)}
            va = emit_vaug(1, 0)
            emit_unit(1, 0, 0, va, ai[(1, 0)])
            emit_proj_mt(0, recv0, range(0, 4))
            emit_unit(1, 0, 1, va, ai[(1, 0)])
            ao[(1, 0)] = emit_a2a(1, 0, ai[(1, 0)])
            emit_proj_mt(0, recv0, range(4, 8))
            emit_recv_head(1, 0, ao[(1, 0)])
            va = emit_vaug(1, 1)
            emit_unit(1, 1, 0, va, ai[(1, 1)])
            emit_unit(1, 1, 1, va, ai[(1, 1)])
            ao[(1, 1)] = emit_a2a(1, 1, ai[(1, 1)])
            recv1 = emit_recv_head(1, 1, ao[(1, 1)])
            emit_proj_mt(1, recv1, range(8))

    _legalize_waits(nc)
    return nc


_NC_CACHE = None


def _get_nc():
    global _NC_CACHE
    if _NC_CACHE is None:
        _NC_CACHE = build_nc()
    return _NC_CACHE


def _make_in_maps(inputs):
    bf = ml_dtypes.bfloat16
    x = np.ascontiguousarray(np.asarray(inputs["x"], dtype=np.float32)).reshape(ROWS, C)
    xt = np.ascontiguousarray(x.T.astype(bf))   # [C, ROWS] bf16
    w_qkv = np.asarray(inputs["w_qkv"], dtype=np.float32)
    b_qkv = np.asarray(inputs["b_qkv"], dtype=np.float32)
    a_q = np.asarray(inputs["a_q"], dtype=np.float32)
    b_q = np.asarray(inputs["b_q"], dtype=np.float32)
    a_v = np.asarray(inputs["a_v"], dtype=np.float32)
    b_v = np.asarray(inputs["b_v"], dtype=np.float32)
    w_proj = np.asarray(inputs["w_proj"], dtype=np.float32)
    b_proj = np.asarray(inputs["b_proj"], dtype=np.float32)

    # fold the (linear) per-head LoRA into the q/v weights and biases:
    # q_final = (x@w_q + b_q) @ (I + a_q@b_q * scale)
    dq = a_q @ b_q * LORA_SCALE                 # [64, 64]
    dv = a_v @ b_v * LORA_SCALE
    mq = np.eye(128, dtype=np.float32)
    mq[0:64, 0:64] += dq
    mq[64:128, 64:128] += dq
    mv = np.eye(128, dtype=np.float32)
    mv[0:64, 0:64] += dv
    mv[64:128, 64:128] += dv

    eye64x2 = np.vstack([np.eye(64, dtype=np.float32)] * 2).astype(bf)

    def warr(w):                                # [1024, n] -> [128, 8*n] chunk-major
        n = w.shape[1]
        return np.ascontiguousarray(
            w.reshape(8, 128, n).transpose(1, 0, 2).reshape(128, 8 * n).astype(bf)
        )

    wp_full = warr(w_proj)                      # [128, 8*1024] bf16
    bp = np.ascontiguousarray(b_proj.reshape(8, 128).T)

    in_maps = []
    for c in range(NCORES):
        wq_c = w_qkv[:, 0 * C + c * PC : 0 * C + (c + 1) * PC] @ mq
        wk_c = w_qkv[:, 1 * C + c * PC : 1 * C + (c + 1) * PC]
        wv_c = w_qkv[:, 2 * C + c * PC : 2 * C + (c + 1) * PC] @ mv
        bq_c = b_qkv[0 * C + c * PC : 0 * C + (c + 1) * PC] @ mq
        bk_c = b_qkv[1 * C + c * PC : 1 * C + (c + 1) * PC]
        bv_c = b_qkv[2 * C + c * PC : 2 * C + (c + 1) * PC] @ mv
        in_maps.append(
            {
                "xt": xt,
                "wq": warr(wq_c),
                "wk": warr(wk_c),
                "wv": warr(wv_c),
                "bq": np.ascontiguousarray(bq_c.reshape(128, 1)),
                "bk": np.ascontiguousarray(bk_c.reshape(128, 1)),
                "bv": np.ascontiguousarray(bv_c.reshape(128, 1)),
                "wp": wp_full,
                "bp": bp,
                "eye64x2": eye64x2,
            }
        )
    return in_maps


def run_sharded(inputs, trace=False, **kw):
    nc = _get_nc()
    in_maps = _make_in_maps(inputs)
    res = run_bass_kernel_spmd(nc, in_maps, list(range(NCORES)), trace=trace, **kw)
    # results[c]["out"]: [B, C, TPC] -- core c's token shard of final y^T
    yT = np.concatenate([res.results[c]["out"] for c in range(NCORES)], axis=2)
    out = np.ascontiguousarray(yT.transpose(0, 2, 1))  # [B, N, C]
    return out, res


def kernel(**inputs) -> np.ndarray:
    out, _ = run_sharded(inputs, trace=False)
    return out


# revision 14
# speedup vs baseline: 1.2381x; 1.0473x over previous
"""Multi-head attention with q/v LoRA on 8 trn2 NeuronCores (bf16 PE path).

Reference computation (B=2, N=2048, C=1024, H=16, HD=64, R=16):
    qkv = x @ w_qkv + b_qkv                -> split per-head q, k, v
    q  += ((q @ a_q) @ b_q) * 2.0          (per head; same for v)
    out = softmax(q k^T / 8) v             (full N x N scores)
    y   = out @ w_proj + b_proj

Sharding: tensor-parallel over heads -- each of the 8 cores owns 2 heads
(128 of the 3*1024 qkv columns) for both batches; attention output is
resharded over tokens with a per-(batch,head) AllToAll so each core
computes final proj rows for its 256 tokens per batch with the full
w_proj.

Key implementation choices:
  * All PE operands are bf16 (hardware runs fp32r at ~2 cycles/row; bf16
    at 1).  PSUM accumulation stays fp32, biases stay fp32.
  * The LoRA is linear in q/v, so it is folded into the qkv weights on
    the host: w_eff = w @ (I + blockdiag(a@b)*scale), same for bias.
    Nothing LoRA-related runs on device.
  * x^T, weights are pre-cast to bf16 on the host and DMA'd straight
    into matmul operand tiles (no on-device rounding copies).
  * Softmax: scores S^T = k^T' q^T -> exp on ACT (bf16 out) -> P @ [v|1]
    in PSUM (ones column gives the row sums).  1/sums via the fast DVE
    reciprocal, broadcast to 64 partitions on the idle GpSimd engine,
    one fused multiply writes normalized bf16 O^T for the AllToAll.
  * v_aug ones columns are memset once into two persistent buffers.
  * AllToAll payloads are bf16 (256 KB per (batch, head)); the receive
    side DMAs the collective output straight into the proj operand tile.
The host stitches the 8 token shards and transposes back to [B, N, C].
"""

import sys

sys.path.insert(0, "/opt/trn_rl_repo")
sys.path.insert(0, "/root/.axon_site")

import numpy as np
import ml_dtypes

import concourse.bass as bass
import concourse.mybir as mybir
import concourse.tile as tile
from concourse.bass_utils import run_bass_kernel_spmd

f32 = mybir.dt.float32
bf16 = mybir.dt.bfloat16
AF = mybir.ActivationFunctionType

B, N, C = 2, 2048, 1024
H, HD, R = 16, 64, 16
LORA_SCALE = 32.0 / R
ATTN_SCALE = HD ** -0.5
NCORES = 8
HPC = H // NCORES          # heads per core = 2
PC = HPC * HD              # partition columns per core = 128
ROWS = B * N               # 4096 tokens
RC = 512                   # row-chunk size for qkv production
TPC = N // NCORES          # tokens per core per batch = 256


def _legalize_waits(nc, max_waits=1):
    """This walrus build accepts at most one sync-wait per instruction;
    Tile attaches several.  Move surplus waits onto same-engine NoOps
    inserted immediately before the instruction (identical semantics)."""
    counter = 0
    for fn in nc.m.functions:
        for bb in fn.blocks:
            insts = bb.instructions
            out = []
            changed = False
            for inst in insts:
                si = inst.sync_info
                if si is not None and si.on_wait and len(si.on_wait) > max_waits:
                    waits = list(si.on_wait)
                    for w in waits[:-max_waits]:
                        counter += 1
                        nop = mybir.InstNoOp(
                            name=f"I-wfix-{counter}",
                            engine=inst.engine,
                            sync_info=mybir.SyncInfo(on_wait=[w], on_update=[]),
                        )
                        nc.register_instruction(nop)
                        out.append(nop)
                    si.on_wait.clear()
                    si.on_wait.extend(waits[-max_waits:])
                    changed = True
                out.append(inst)
            if changed:
                insts[:] = out


def build_nc():
    nc = bass.Bass(num_devices=NCORES)

    xt_d = nc.dram_tensor("xt", [C, ROWS], bf16, kind="ExternalInput")
    wq_d = nc.dram_tensor("wq", [128, 1024], bf16, kind="ExternalInput")
    wk_d = nc.dram_tensor("wk", [128, 1024], bf16, kind="ExternalInput")
    wv_d = nc.dram_tensor("wv", [128, 1024], bf16, kind="ExternalInput")
    bq_d = nc.dram_tensor("bq", [128, 1], f32, kind="ExternalInput")
    bk_d = nc.dram_tensor("bk", [128, 1], f32, kind="ExternalInput")
    bv_d = nc.dram_tensor("bv", [128, 1], f32, kind="ExternalInput")
    wp_d = nc.dram_tensor("wp", [128, 8 * 1024], bf16, kind="ExternalInput")
    bp_d = nc.dram_tensor("bp", [128, 8], f32, kind="ExternalInput")
    eye64x2_d = nc.dram_tensor("eye64x2", [128, 64], bf16, kind="ExternalInput")
    out_d = nc.dram_tensor("out", [B, C, TPC], f32, kind="ExternalOutput")

    with nc.allow_low_precision(
        reason="bf16 matmul operands are intended; PSUM accumulation stays fp32"
    ), tile.TileContext(nc) as tc:
        with (
            tc.tile_pool(name="persist", bufs=1) as persist,
            tc.tile_pool(name="const", bufs=1) as const,
            tc.tile_pool(name="dram", bufs=1, space="DRAM") as dram,
            tc.tile_pool(name="xio", bufs=2) as xio_p,
            tc.tile_pool(name="work", bufs=2) as work_p,
            tc.tile_pool(name="ps", bufs=1, space="PSUM") as ps,
        ):
            qT = persist.tile([128, ROWS], bf16, tag="qT", name="qT")
            kT = persist.tile([128, ROWS], bf16, tag="kT", name="kT")
            vT = persist.tile([128, ROWS], bf16, tag="vT", name="vT")

            # prefetch the first x^T chunk ahead of the weight DMAs
            xT0 = xio_p.tile([128, 8 * RC], bf16, tag="xT", name="xT00")
            nc.sync.dma_start(
                out=xT0[:].rearrange("p (a r) -> p a r", a=8),
                in_=xt_d[:, 0:RC].rearrange("(a p) r -> p a r", p=128),
            )

            w_t = []
            for nm, d in (("wq", wq_d), ("wk", wk_d), ("wv", wv_d)):
                t = const.tile([128, 1024], bf16, tag=nm, name=f"{nm}_t")
                nc.sync.dma_start(out=t[:], in_=d[:])
                w_t.append(t)

            eye64x2 = const.tile([128, 64], bf16, tag="eye64", name="eye64")
            nc.sync.dma_start(out=eye64x2[:], in_=eye64x2_d[:])

            bias_t = []
            for nm, d in (("bq", bq_d), ("bk", bk_d), ("bv", bv_d)):
                bt = const.tile([128, 1], f32, tag=nm, name=f"{nm}_t")
                nc.sync.dma_start(out=bt[:], in_=d[:])
                bias_t.append(bt)
            bp_t = const.tile([128, 8], f32, tag="bp", name="bp_t")
            nc.sync.dma_start(out=bp_t[:], in_=bp_d[:])

            wp_t = const.tile([128, 8 * 1024], bf16, tag="wp_t", name="wp_t")

            # persistent v_aug buffers (one per (batch, head)): ones columns
            # written once by memset, data blocks overwritten by transposes
            v_aug_bufs = {}
            for b in range(B):
                for hl in range(HPC):
                    va = persist.tile(
                        [128, 16 * 65], bf16, tag=f"va{b}{hl}", name=f"va{b}{hl}"
                    )
                    nc.gpsimd.memset(va[:], 1.0)
                    v_aug_bufs[(b, hl)] = va

            ones_row = const.tile([1, 64], bf16, tag="ones_r", name="ones_r")
            nc.gpsimd.memset(ones_row[:], 1.0)

            qkvT = (qT, kT, vT)

            def emit_qkv_chunk(b, rci, xT_t=None, act_ok=True):
                r0 = b * N + rci * RC
                if xT_t is None:
                    xT_t = xio_p.tile([128, 8 * RC], bf16, tag="xT", name=f"xT{b}{rci}")
                    nc.sync.dma_start(
                        out=xT_t[:].rearrange("p (a r) -> p a r", a=8),
                        in_=xt_d[:, r0 : r0 + RC].rearrange("(a p) r -> p a r", p=128),
                    )
                for m in range(3):
                    acc = ps.tile([128, RC], f32, tag="acc", bufs=2, name=f"ac{b}{rci}{m}")
                    for ci in range(8):
                        nc.tensor.matmul(
                            acc[:],
                            w_t[m][:, ci * 128 : (ci + 1) * 128],
                            xT_t[:, ci * RC : (ci + 1) * RC],
                            start=(ci == 0),
                            stop=(ci == 7),
                        )
                    dst = qkvT[m][:, r0 : r0 + RC]
                    if m == 0 and act_ok:
                        nc.scalar.activation(dst, acc[:], AF.Identity, bias=bias_t[m][:])
                    else:
                        nc.vector.tensor_scalar_add(dst, acc[:], bias_t[m][:])

            def emit_vaug(b, hl):
                boff = b * N
                hs = slice(hl * HD, (hl + 1) * HD)
                v_aug = v_aug_bufs[(b, hl)]
                for kt4 in range(4):
                    vtr = ps.tile([128, 256], bf16, tag="s", bufs=2, name=f"vt{b}{hl}{kt4}")
                    for j in range(4):
                        ko = boff + (kt4 * 4 + j) * 128
                        nc.tensor.transpose(
                            vtr[:, j * 64 : (j + 1) * 64],
                            vT[hs, ko : ko + 128],
                            eye64x2[hs, :],
                        )
                    nc.vector.tensor_copy(
                        v_aug[:].rearrange("p (k e) -> p k e", e=65)[
                            :, kt4 * 4 : kt4 * 4 + 4, 0:64
                        ],
                        vtr[:].rearrange("p (k e) -> p k e", e=64),
                    )
                return v_aug

            # filler queue: small batches of independent PE work emitted
            # between attention kt iterations so the tensor engine never
            # starves while waiting on the ACT exp cadence
            filler_q = []

            def pop_filler():
                if filler_q:
                    filler_q.pop(0)()

            def emit_unit(b, hl, qh, a2a_in):
                """Emit scores/exp/PV for one (batch, head, q-half) unit.
                Returns a finisher closure (normalize + a2a staging DMA) to
                be emitted later -- after the next unit's first matmuls -- so
                the slow reciprocal never blocks the in-order PE queue."""
                boff = b * N
                hs = slice(hl * HD, (hl + 1) * HD)
                qoff = boff + qh * 1024
                v_aug = v_aug_bufs[(b, hl)]
                o_ps = ps.tile([65, 1024], f32, tag="o", bufs=1, name=f"o{b}{hl}{qh}")

                def emit_pv(p_tile, kt):
                    for qc in range(2):
                        nc.tensor.matmul(
                            o_ps[:, qc * 512 : (qc + 1) * 512],
                            v_aug[:, kt * 65 : kt * 65 + 65],
                            p_tile[:, qc * 512 : (qc + 1) * 512],
                            start=(kt == 0),
                            stop=(kt == 15),
                        )

                pending = None
                for kt in range(16):
                    ko = boff + kt * 128
                    s_ps = ps.tile([128, 1024], f32, tag="s", bufs=2, name=f"s{b}{hl}{qh}{kt}")
                    for qc in range(2):
                        nc.tensor.matmul(
                            s_ps[:, qc * 512 : (qc + 1) * 512],
                            kT[hs, ko : ko + 128],
                            qT[hs, qoff + qc * 512 : qoff + (qc + 1) * 512],
                            start=True,
                            stop=True,
                        )
                    p_sb = work_p.tile([128, 1024], bf16, tag="p", bufs=3, name=f"p{qh}{kt}")
                    nc.scalar.activation(p_sb[:], s_ps[:], AF.Exp, scale=ATTN_SCALE)
                    if pending is not None:
                        emit_pv(*pending)
                        if kt % 2 == 0:
                            pop_filler()
                    pending = (p_sb, kt)
                emit_pv(*pending)
                # DVE-side epilogue now (doesn't touch the PE queue): copy
                # O^T + sums out of PSUM, take reciprocals per q-half
                nst = work_p.tile([65, 1024], f32, tag="nst", bufs=2, name=f"n{hl}{qh}")
                nc.vector.tensor_copy(nst[:], o_ps[:])
                r_bf = work_p.tile([1, 1024], bf16, tag="rbf", bufs=2, name=f"rb{b}{hl}{qh}")
                for qc in range(2):
                    r_sb = work_p.tile([1, 512], f32, tag="r", bufs=2, name=f"r{b}{hl}{qh}{qc}")
                    nc.vector.reciprocal(r_sb[:], nst[64:65, qc * 512 : (qc + 1) * 512])
                    nc.vector.tensor_copy(r_bf[:, qc * 512 : (qc + 1) * 512], r_sb[:])

                def finisher():
                    onrm = work_p.tile(
                        [64, 1024], bf16, tag="onrm", bufs=2, name=f"on{b}{hl}{qh}"
                    )
                    for qc in range(2):
                        bc_ps = ps.tile([64, 512], f32, tag="acc", bufs=2, name=f"bc{qc}")
                        nc.tensor.matmul(
                            bc_ps[:],
                            ones_row[:],
                            r_bf[:, qc * 512 : (qc + 1) * 512],
                            start=True,
                            stop=True,
                        )
                        nc.vector.tensor_mul(
                            onrm[:, qc * 512 : (qc + 1) * 512],
                            nst[0:64, qc * 512 : (qc + 1) * 512],
                            bc_ps[:],
                        )
                    nc.sync.dma_start(
                        out=a2a_in[qh * 4 : (qh + 1) * 4, :, :].rearrange(
                            "a p r -> p a r"
                        ),
                        in_=onrm[:].rearrange("p (a r) -> p a r", a=4),
                    )

                return finisher

            def emit_a2a(b, hl, a2a_in):
                a2a_out = dram.tile(
                    [8, 64, TPC], bf16, tag=f"ao{b}{hl}", name=f"ao{b}{hl}"
                )
                nc.gpsimd.collective_compute(
                    "AllToAll",
                    mybir.AluOpType.bypass,
                    replica_groups=[list(range(NCORES))],
                    ins=[a2a_in[:].opt()],
                    outs=[a2a_out[:].opt()],
                )
                return a2a_out

            def new_a2a_in(b, hl):
                return dram.tile([8, 64, TPC], bf16, tag=f"ai{b}{hl}", name=f"ai{b}{hl}")

            recv_tiles = {}

            def get_recv(b):
                if b not in recv_tiles:
                    recv_tiles[b] = work_p.tile(
                        [128, 8 * TPC], bf16, tag=f"rcr{b}", bufs=1, name=f"rr{b}"
                    )
                return recv_tiles[b]

            def emit_recv_head(b, hl, a2a_out):
                recv_r = get_recv(b)
                nc.sync.dma_start(
                    out=recv_r[hl * 64 : (hl + 1) * 64, :].rearrange(
                        "p (a r) -> p a r", a=8
                    ),
                    in_=a2a_out[:].rearrange("a p r -> p a r"),
                )
                return recv_r

            def emit_proj_mt(b, recv_r, mt):
                y_ps = ps.tile([128, TPC], f32, tag="acc", bufs=2, name=f"y{b}{mt}")
                for kc in range(8):
                    nc.tensor.matmul(
                        y_ps[:],
                        wp_t[:, kc * 1024 + mt * 128 : kc * 1024 + (mt + 1) * 128],
                        recv_r[:, kc * TPC : (kc + 1) * TPC],
                        start=(kc == 0),
                        stop=(kc == 7),
                    )
                yst = work_p.tile([128, TPC], f32, tag="yst", bufs=3, name=f"ys{b}{mt}")
                nc.vector.tensor_scalar_add(yst[:], y_ps[:], bp_t[:, mt : mt + 1])
                nc.sync.dma_start(
                    out=out_d[b, mt * 128 : (mt + 1) * 128, :], in_=yst[:]
                )

            def emit_proj_pass(b, recv_r, mt, y_ap, hl, last):
                """K=64 proj pass over one head-pair block; accumulates into
                y_ap across the two passes (hl=0 start, hl=1 stop)."""
                lo, hi = hl * 64, (hl + 1) * 64
                for kc in range(8):
                    nc.tensor.matmul(
                        y_ap,
                        wp_t[lo:hi, kc * 1024 + mt * 128 : kc * 1024 + (mt + 1) * 128],
                        recv_r[lo:hi, kc * TPC : (kc + 1) * TPC],
                        start=(hl == 0 and kc == 0),
                        stop=(last and kc == 7),
                        skip_group_check=True,
                    )
                if last:
                    yst = work_p.tile(
                        [128, TPC], f32, tag="yst", bufs=3, name=f"ys{b}{mt}"
                    )
                    nc.vector.tensor_scalar_add(yst[:], y_ap, bp_t[:, mt : mt + 1])
                    nc.sync.dma_start(
                        out=out_d[b, mt * 128 : (mt + 1) * 128, :], in_=yst[:]
                    )

            # ---- emission schedule ---------------------------------------
            # Phase 1: all qkv chunks back-to-back (dense PE stream ramps the
            # clock), then all four v_aug transpose blocks.
            emit_qkv_chunk(0, 0, xT_t=xT0)
            for rci in range(1, 4):
                emit_qkv_chunk(0, rci)
            for rci in range(4):
                emit_qkv_chunk(1, rci)
            nc.sync.dma_start(out=wp_t[:], in_=wp_d[:])
            for b in range(B):
                for hl in range(HPC):
                    emit_vaug(b, hl)

            # Phase 2: attention units; each unit's normalize runs as filler
            # inside the next unit, as do proj(0) column blocks.
            ai = {
                (b, hl): new_a2a_in(b, hl) for b in range(B) for hl in range(HPC)
            }
            ao = {}
            recv0 = get_recv(0)
            recv1 = get_recv(1)

            fin = emit_unit(0, 0, 0, ai[(0, 0)])
            filler_q.append(fin)
            fin = emit_unit(0, 0, 1, ai[(0, 0)])
            filler_q.append(fin)
            filler_q.append(lambda: ao.update({(0, 0): emit_a2a(0, 0, ai[(0, 0)])}))
            fin = emit_unit(0, 1, 0, ai[(0, 1)])
            filler_q.append(fin)
            fin = emit_unit(0, 1, 1, ai[(0, 1)])
            filler_q.append(fin)
            filler_q.append(lambda: ao.update({(0, 1): emit_a2a(0, 1, ai[(0, 1)])}))
            filler_q.append(lambda: emit_recv_head(0, 0, ao[(0, 0)]))
            fin = emit_unit(1, 0, 0, ai[(1, 0)])
            filler_q.append(fin)
            filler_q.append(lambda: emit_recv_head(0, 1, ao[(0, 1)]))
            fin = emit_unit(1, 0, 1, ai[(1, 0)])
            filler_q.append(fin)
            filler_q.append(lambda: ao.update({(1, 0): emit_a2a(1, 0, ai[(1, 0)])}))
            for mt in range(4):
                filler_q.append(lambda mt=mt: emit_proj_mt(0, recv0, mt))
            fin = emit_unit(1, 1, 0, ai[(1, 1)])
            filler_q.append(fin)
            filler_q.append(lambda: emit_recv_head(1, 0, ao[(1, 0)]))
            for mt in range(4, 8):
                filler_q.append(lambda mt=mt: emit_proj_mt(0, recv0, mt))
            fin = emit_unit(1, 1, 1, ai[(1, 1)])
            while filler_q:
                pop_filler()
            fin()
            ao[(1, 1)] = emit_a2a(1, 1, ai[(1, 1)])

            # Phase 3: proj(1) in two K=64 passes -- pass A (head block 0)
            # overlaps the in-flight a2a(1,1); pass B after its receive.
            # PSUM matmul start=True resets the whole 2 KB bank, so every mt
            # accumulator must own a distinct bank: spread the 8 groups over
            # the (now idle) s/o/acc pool slots, one group per bank.
            y1a = ps.tile([128, 1024], f32, tag="s", bufs=2, name="y1a")
            y1b = ps.tile([128, 1024], f32, tag="s", bufs=2, name="y1b")
            y1c = ps.tile([128, 1024], f32, tag="o", bufs=1, name="y1c")
            y1d = ps.tile([128, 512], f32, tag="acc", bufs=2, name="y1d")
            y1e = ps.tile([128, 512], f32, tag="acc", bufs=2, name="y1e")

            def y_ap(mt):
                # mts 0-5: two per two-bank tile, one per bank (cols 0-255
                # in bank 0, cols 512-767 in bank 1); mts 6-7: one-bank tiles
                if mt < 6:
                    t = (y1a, y1b, y1c)[mt // 2]
                    return t[:, (mt % 2) * 512 : (mt % 2) * 512 + TPC]
                t = y1d if mt == 6 else y1e
                return t[:, 0:TPC]

            for mt in range(8):
                emit_proj_pass(1, recv1, mt, y_ap(mt), 0, last=False)
            emit_recv_head(1, 1, ao[(1, 1)])
            for mt in range(8):
                emit_proj_pass(1, recv1, mt, y_ap(mt), 1, last=True)

    _legalize_waits(nc)
    return nc


_NC_CACHE = None


def _get_nc():
    global _NC_CACHE
    if _NC_CACHE is None:
        _NC_CACHE = build_nc()
    return _NC_CACHE


def _make_in_maps(inputs):
    bf = ml_dtypes.bfloat16
    x = np.ascontiguousarray(np.asarray(inputs["x"], dtype=np.float32)).reshape(ROWS, C)
    xt = np.ascontiguousarray(x.T.astype(bf))   # [C, ROWS] bf16
    w_qkv = np.asarray(inputs["w_qkv"], dtype=np.float32)
    b_qkv = np.asarray(inputs["b_qkv"], dtype=np.float32)
    a_q = np.asarray(inputs["a_q"], dtype=np.float32)
    b_q = np.asarray(inputs["b_q"], dtype=np.float32)
    a_v = np.asarray(inputs["a_v"], dtype=np.float32)
    b_v = np.asarray(inputs["b_v"], dtype=np.float32)
    w_proj = np.asarray(inputs["w_proj"], dtype=np.float32)
    b_proj = np.asarray(inputs["b_proj"], dtype=np.float32)

    # fold the (linear) per-head LoRA into the q/v weights and biases:
    # q_final = (x@w_q + b_q) @ (I + a_q@b_q * scale)
    dq = a_q @ b_q * LORA_SCALE                 # [64, 64]
    dv = a_v @ b_v * LORA_SCALE
    mq = np.eye(128, dtype=np.float32)
    mq[0:64, 0:64] += dq
    mq[64:128, 64:128] += dq
    mv = np.eye(128, dtype=np.float32)
    mv[0:64, 0:64] += dv
    mv[64:128, 64:128] += dv

    eye64x2 = np.vstack([np.eye(64, dtype=np.float32)] * 2).astype(bf)

    def warr(w):                                # [1024, n] -> [128, 8*n] chunk-major
        n = w.shape[1]
        return np.ascontiguousarray(
            w.reshape(8, 128, n).transpose(1, 0, 2).reshape(128, 8 * n).astype(bf)
        )

    wp_full = warr(w_proj)                      # [128, 8*1024] bf16
    bp = np.ascontiguousarray(b_proj.reshape(8, 128).T)

    in_maps = []
    for c in range(NCORES):
        wq_c = w_qkv[:, 0 * C + c * PC : 0 * C + (c + 1) * PC] @ mq
        wk_c = w_qkv[:, 1 * C + c * PC : 1 * C + (c + 1) * PC]
        wv_c = w_qkv[:, 2 * C + c * PC : 2 * C + (c + 1) * PC] @ mv
        bq_c = b_qkv[0 * C + c * PC : 0 * C + (c + 1) * PC] @ mq
        bk_c = b_qkv[1 * C + c * PC : 1 * C + (c + 1) * PC]
        bv_c = b_qkv[2 * C + c * PC : 2 * C + (c + 1) * PC] @ mv
        in_maps.append(
            {
                "xt": xt,
                "wq": warr(wq_c),
                "wk": warr(wk_c),
                "wv": warr(wv_c),
                "bq": np.ascontiguousarray(bq_c.reshape(128, 1)),
                "bk": np.ascontiguousarray(bk_c.reshape(128, 1)),
                "bv": np.ascontiguousarray(bv_c.reshape(128, 1)),
                "wp": wp_full,
                "bp": bp,
                "eye64x2": eye64x2,
            }
        )
    return in_maps


def run_sharded(inputs, trace=False, **kw):
    nc = _get_nc()
    in_maps = _make_in_maps(inputs)
    res = run_bass_kernel_spmd(nc, in_maps, list(range(NCORES)), trace=trace, **kw)
    # results[c]["out"]: [B, C, TPC] -- core c's token shard of final y^T
    yT = np.concatenate([res.results[c]["out"] for c in range(NCORES)], axis=2)
    out = np.ascontiguousarray(yT.transpose(0, 2, 1))  # [B, N, C]
    return out, res


def kernel(**inputs) -> np.ndarray:
    out, _ = run_sharded(inputs, trace=False)
    return out


# revision 17
# speedup vs baseline: 1.3234x; 1.0689x over previous
"""Multi-head attention with q/v LoRA on 8 trn2 NeuronCores (bf16 PE path).

Reference computation (B=2, N=2048, C=1024, H=16, HD=64, R=16):
    qkv = x @ w_qkv + b_qkv                -> split per-head q, k, v
    q  += ((q @ a_q) @ b_q) * 2.0          (per head; same for v)
    out = softmax(q k^T / 8) v             (full N x N scores)
    y   = out @ w_proj + b_proj

Sharding: tensor-parallel over heads -- each of the 8 cores owns 2 heads
(128 of the 3*1024 qkv columns) for both batches; attention output is
resharded over tokens with a per-(batch,head) AllToAll so each core
computes final proj rows for its 256 tokens per batch with the full
w_proj.

Key implementation choices:
  * All PE operands are bf16 (hardware runs fp32r at ~2 cycles/row; bf16
    at 1).  PSUM accumulation stays fp32, biases stay fp32.
  * The LoRA is linear in q/v, so it is folded into the qkv weights on
    the host: w_eff = w @ (I + blockdiag(a@b)*scale), same for bias.
    Nothing LoRA-related runs on device.
  * x^T, weights are pre-cast to bf16 on the host and DMA'd straight
    into matmul operand tiles (no on-device rounding copies).
  * Softmax: scores S^T = k^T' q^T -> exp on ACT (bf16 out) -> P @ [v|1]
    in PSUM (ones column gives the row sums).  1/sums via the fast DVE
    reciprocal, broadcast to 64 partitions on the idle GpSimd engine,
    one fused multiply writes normalized bf16 O^T for the AllToAll.
  * v_aug ones columns are memset once into two persistent buffers.
  * AllToAll payloads are bf16 (256 KB per (batch, head)); the receive
    side DMAs the collective output straight into the proj operand tile.
The host stitches the 8 token shards and transposes back to [B, N, C].
"""

import sys

sys.path.insert(0, "/opt/trn_rl_repo")
sys.path.insert(0, "/root/.axon_site")

import numpy as np
import ml_dtypes

import concourse.bass as bass
import concourse.mybir as mybir
import concourse.tile as tile
from concourse.bass_utils import run_bass_kernel_spmd

f32 = mybir.dt.float32
bf16 = mybir.dt.bfloat16
AF = mybir.ActivationFunctionType

B, N, C = 2, 2048, 1024
H, HD, R = 16, 64, 16
LORA_SCALE = 32.0 / R
ATTN_SCALE = HD ** -0.5
NCORES = 8
HPC = H // NCORES          # heads per core = 2
PC = HPC * HD              # partition columns per core = 128
ROWS = B * N               # 4096 tokens
RC = 512                   # row-chunk size for qkv production
TPC = N // NCORES          # tokens per core per batch = 256


def _legalize_waits(nc, max_waits=1):
    """This walrus build accepts at most one sync-wait per instruction;
    Tile attaches several.  Move surplus waits onto same-engine NoOps
    inserted immediately before the instruction (identical semantics)."""
    counter = 0
    for fn in nc.m.functions:
        for bb in fn.blocks:
            insts = bb.instructions
            out = []
            changed = False
            for inst in insts:
                si = inst.sync_info
                if si is not None and si.on_wait and len(si.on_wait) > max_waits:
                    waits = list(si.on_wait)
                    for w in waits[:-max_waits]:
                        counter += 1
                        nop = mybir.InstNoOp(
                            name=f"I-wfix-{counter}",
                            engine=inst.engine,
                            sync_info=mybir.SyncInfo(on_wait=[w], on_update=[]),
                        )
                        nc.register_instruction(nop)
                        out.append(nop)
                    si.on_wait.clear()
                    si.on_wait.extend(waits[-max_waits:])
                    changed = True
                out.append(inst)
            if changed:
                insts[:] = out


def build_nc():
    nc = bass.Bass(num_devices=NCORES)

    xt_d = nc.dram_tensor("xt", [C, ROWS], bf16, kind="ExternalInput")
    wq_d = nc.dram_tensor("wq", [128, 1024], bf16, kind="ExternalInput")
    wk_d = nc.dram_tensor("wk", [128, 1024], bf16, kind="ExternalInput")
    wv_d = nc.dram_tensor("wv", [128, 1024], bf16, kind="ExternalInput")
    bq_d = nc.dram_tensor("bq", [128, 1], f32, kind="ExternalInput")
    bk_d = nc.dram_tensor("bk", [128, 1], f32, kind="ExternalInput")
    bv_d = nc.dram_tensor("bv", [128, 1], f32, kind="ExternalInput")
    wp_d = nc.dram_tensor("wp", [128, 8 * 1024], bf16, kind="ExternalInput")
    bp_d = nc.dram_tensor("bp", [128, 8], f32, kind="ExternalInput")
    eye64x2_d = nc.dram_tensor("eye64x2", [128, 64], bf16, kind="ExternalInput")
    out_d = nc.dram_tensor("out", [B, C, TPC], f32, kind="ExternalOutput")

    with nc.allow_low_precision(
        reason="bf16 matmul operands are intended; PSUM accumulation stays fp32"
    ), tile.TileContext(nc) as tc:
        with (
            tc.tile_pool(name="persist", bufs=1) as persist,
            tc.tile_pool(name="const", bufs=1) as const,
            tc.tile_pool(name="dram", bufs=1, space="DRAM") as dram,
            tc.tile_pool(name="xio", bufs=2) as xio_p,
            tc.tile_pool(name="work", bufs=2) as work_p,
            tc.tile_pool(name="ps", bufs=1, space="PSUM") as ps,
        ):
            qT = persist.tile([128, ROWS], bf16, tag="qT", name="qT")
            kT = persist.tile([128, ROWS], bf16, tag="kT", name="kT")
            vT = persist.tile([128, ROWS], bf16, tag="vT", name="vT")

            # prefetch the first x^T chunk ahead of the weight DMAs
            xT0 = xio_p.tile([128, 8 * RC], bf16, tag="xT", bufs=6, name="xT00")
            nc.sync.dma_start(
                out=xT0[:].rearrange("p (a r) -> p a r", a=8),
                in_=xt_d[:, 0:RC].rearrange("(a p) r -> p a r", p=128),
            )

            w_t = []
            for nm, d in (("wq", wq_d), ("wk", wk_d), ("wv", wv_d)):
                t = const.tile([128, 1024], bf16, tag=nm, name=f"{nm}_t")
                nc.sync.dma_start(out=t[:], in_=d[:])
                w_t.append(t)

            eye64x2 = const.tile([128, 64], bf16, tag="eye64", name="eye64")
            nc.sync.dma_start(out=eye64x2[:], in_=eye64x2_d[:])

            bias_t = []
            for nm, d in (("bq", bq_d), ("bk", bk_d), ("bv", bv_d)):
                bt = const.tile([128, 1], f32, tag=nm, name=f"{nm}_t")
                nc.sync.dma_start(out=bt[:], in_=d[:])
                bias_t.append(bt)
            bp_t = const.tile([128, 8], f32, tag="bp", name="bp_t")
            nc.sync.dma_start(out=bp_t[:], in_=bp_d[:])

            wp_t = const.tile([128, 8 * 1024], bf16, tag="wp_t", name="wp_t")

            # persistent v_aug buffers (one per (batch, head)): ones columns
            # written once by memset, data blocks overwritten by transposes
            v_aug_bufs = {}
            for b in range(B):
                for hl in range(HPC):
                    va = persist.tile(
                        [128, 16 * 65], bf16, tag=f"va{b}{hl}", name=f"va{b}{hl}"
                    )
                    nc.gpsimd.memset(va[:], 1.0)
                    v_aug_bufs[(b, hl)] = va

            ones_row = const.tile([1, 64], bf16, tag="ones_r", name="ones_r")
            nc.gpsimd.memset(ones_row[:], 1.0)

            qkvT = (qT, kT, vT)

            xts = {}

            def load_xchunk(b, rci):
                r0 = b * N + rci * RC
                xT_t = xio_p.tile(
                    [128, 8 * RC], bf16, tag="xT", bufs=6, name=f"xT{b}{rci}"
                )
                nc.sync.dma_start(
                    out=xT_t[:].rearrange("p (a r) -> p a r", a=8),
                    in_=xt_d[:, r0 : r0 + RC].rearrange("(a p) r -> p a r", p=128),
                )
                xts[(b, rci)] = xT_t
                return xT_t

            def emit_qkv_m(b, rci, m, act_ok=True):
                r0 = b * N + rci * RC
                xT_t = xts[(b, rci)]
                acc = ps.tile([128, RC], f32, tag="acc", bufs=2, name=f"ac{b}{rci}{m}")
                for ci in range(8):
                    nc.tensor.matmul(
                        acc[:],
                        w_t[m][:, ci * 128 : (ci + 1) * 128],
                        xT_t[:, ci * RC : (ci + 1) * RC],
                        start=(ci == 0),
                        stop=(ci == 7),
                    )
                dst = qkvT[m][:, r0 : r0 + RC]
                if m == 0 and act_ok:
                    nc.scalar.activation(dst, acc[:], AF.Identity, bias=bias_t[m][:])
                else:
                    nc.vector.tensor_scalar_add(dst, acc[:], bias_t[m][:])

            def emit_qkv_chunk(b, rci, xT_t=None, act_ok=True):
                if (b, rci) not in xts:
                    if xT_t is not None:
                        xts[(b, rci)] = xT_t
                    else:
                        load_xchunk(b, rci)
                for m in range(3):
                    emit_qkv_m(b, rci, m, act_ok=act_ok)

            def emit_vaug(b, hl):
                boff = b * N
                hs = slice(hl * HD, (hl + 1) * HD)
                v_aug = v_aug_bufs[(b, hl)]
                for kt4 in range(4):
                    vtr = ps.tile([128, 256], bf16, tag="s", bufs=2, name=f"vt{b}{hl}{kt4}")
                    for j in range(4):
                        ko = boff + (kt4 * 4 + j) * 128
                        nc.tensor.transpose(
                            vtr[:, j * 64 : (j + 1) * 64],
                            vT[hs, ko : ko + 128],
                            eye64x2[hs, :],
                        )
                    nc.vector.tensor_copy(
                        v_aug[:].rearrange("p (k e) -> p k e", e=65)[
                            :, kt4 * 4 : kt4 * 4 + 4, 0:64
                        ],
                        vtr[:].rearrange("p (k e) -> p k e", e=64),
                    )
                return v_aug

            # filler queue: small batches of independent PE work emitted
            # between attention kt iterations so the tensor engine never
            # starves while waiting on the ACT exp cadence
            filler_q = []

            def pop_filler():
                if filler_q:
                    filler_q.pop(0)()

            def emit_unit(b, hl, qh, a2a_in):
                """Emit scores/exp/PV for one (batch, head, q-half) unit.
                Returns a finisher closure (normalize + a2a staging DMA) to
                be emitted later -- after the next unit's first matmuls -- so
                the slow reciprocal never blocks the in-order PE queue."""
                boff = b * N
                hs = slice(hl * HD, (hl + 1) * HD)
                qoff = boff + qh * 1024
                v_aug = v_aug_bufs[(b, hl)]
                o_ps = ps.tile([65, 1024], f32, tag="o", bufs=1, name=f"o{b}{hl}{qh}")

                def emit_pv(p_tile, kt):
                    for qc in range(2):
                        nc.tensor.matmul(
                            o_ps[:, qc * 512 : (qc + 1) * 512],
                            v_aug[:, kt * 65 : kt * 65 + 65],
                            p_tile[:, qc * 512 : (qc + 1) * 512],
                            start=(kt == 0),
                            stop=(kt == 15),
                        )

                pending = None
                for kt in range(16):
                    ko = boff + kt * 128
                    s_ps = ps.tile([128, 1024], f32, tag="s", bufs=2, name=f"s{b}{hl}{qh}{kt}")
                    for qc in range(2):
                        nc.tensor.matmul(
                            s_ps[:, qc * 512 : (qc + 1) * 512],
                            kT[hs, ko : ko + 128],
                            qT[hs, qoff + qc * 512 : qoff + (qc + 1) * 512],
                            start=True,
                            stop=True,
                        )
                    p_sb = work_p.tile([128, 1024], bf16, tag="p", bufs=3, name=f"p{qh}{kt}")
                    nc.scalar.activation(p_sb[:], s_ps[:], AF.Exp, scale=ATTN_SCALE)
                    if pending is not None:
                        emit_pv(*pending)
                        if kt % 2 == 0:
                            pop_filler()
                    pending = (p_sb, kt)
                emit_pv(*pending)
                # DVE-side epilogue now (doesn't touch the PE queue): copy
                # O^T + sums out of PSUM, take reciprocals per q-half
                nst = work_p.tile([65, 1024], f32, tag="nst", bufs=2, name=f"n{hl}{qh}")
                nc.vector.tensor_copy(nst[:], o_ps[:])
                r_bf = work_p.tile([1, 1024], bf16, tag="rbf", bufs=2, name=f"rb{b}{hl}{qh}")
                for qc in range(2):
                    r_sb = work_p.tile([1, 512], f32, tag="r", bufs=2, name=f"r{b}{hl}{qh}{qc}")
                    nc.vector.reciprocal(r_sb[:], nst[64:65, qc * 512 : (qc + 1) * 512])
                    nc.vector.tensor_copy(r_bf[:, qc * 512 : (qc + 1) * 512], r_sb[:])

                def finisher():
                    onrm = work_p.tile(
                        [64, 1024], bf16, tag="onrm", bufs=2, name=f"on{b}{hl}{qh}"
                    )
                    for qc in range(2):
                        bc_ps = ps.tile([64, 512], f32, tag="acc", bufs=2, name=f"bc{qc}")
                        nc.tensor.matmul(
                            bc_ps[:],
                            ones_row[:],
                            r_bf[:, qc * 512 : (qc + 1) * 512],
                            start=True,
                            stop=True,
                        )
                        nc.vector.tensor_mul(
                            onrm[:, qc * 512 : (qc + 1) * 512],
                            nst[0:64, qc * 512 : (qc + 1) * 512],
                            bc_ps[:],
                        )
                    nc.sync.dma_start(
                        out=a2a_in[qh * 4 : (qh + 1) * 4, :, :].rearrange(
                            "a p r -> p a r"
                        ),
                        in_=onrm[:].rearrange("p (a r) -> p a r", a=4),
                    )

                return finisher

            def emit_a2a(b, hl, a2a_in):
                a2a_out = dram.tile(
                    [8, 64, TPC], bf16, tag=f"ao{b}{hl}", name=f"ao{b}{hl}"
                )
                nc.gpsimd.collective_compute(
                    "AllToAll",
                    mybir.AluOpType.bypass,
                    replica_groups=[list(range(NCORES))],
                    ins=[a2a_in[:].opt()],
                    outs=[a2a_out[:].opt()],
                )
                return a2a_out

            def new_a2a_in(b, hl):
                return dram.tile([8, 64, TPC], bf16, tag=f"ai{b}{hl}", name=f"ai{b}{hl}")

            recv_tiles = {}

            def get_recv(b):
                if b not in recv_tiles:
                    recv_tiles[b] = work_p.tile(
                        [128, 8 * TPC], bf16, tag=f"rcr{b}", bufs=1, name=f"rr{b}"
                    )
                return recv_tiles[b]

            def emit_recv_head(b, hl, a2a_out):
                recv_r = get_recv(b)
                nc.sync.dma_start(
                    out=recv_r[hl * 64 : (hl + 1) * 64, :].rearrange(
                        "p (a r) -> p a r", a=8
                    ),
                    in_=a2a_out[:].rearrange("a p r -> p a r"),
                )
                return recv_r

            def emit_proj_mt(b, recv_r, mt):
                y_ps = ps.tile([128, TPC], f32, tag="acc", bufs=2, name=f"y{b}{mt}")
                for kc in range(8):
                    nc.tensor.matmul(
                        y_ps[:],
                        wp_t[:, kc * 1024 + mt * 128 : kc * 1024 + (mt + 1) * 128],
                        recv_r[:, kc * TPC : (kc + 1) * TPC],
                        start=(kc == 0),
                        stop=(kc == 7),
                    )
                yst = work_p.tile([128, TPC], f32, tag="yst", bufs=3, name=f"ys{b}{mt}")
                nc.vector.tensor_scalar_add(yst[:], y_ps[:], bp_t[:, mt : mt + 1])
                nc.sync.dma_start(
                    out=out_d[b, mt * 128 : (mt + 1) * 128, :], in_=yst[:]
                )

            def emit_proj_pass(b, recv_r, mt, y_ap, hl, last):
                """K=64 proj pass over one head-pair block; accumulates into
                y_ap across the two passes (hl=0 start, hl=1 stop)."""
                lo, hi = hl * 64, (hl + 1) * 64
                for kc in range(8):
                    nc.tensor.matmul(
                        y_ap,
                        wp_t[lo:hi, kc * 1024 + mt * 128 : kc * 1024 + (mt + 1) * 128],
                        recv_r[lo:hi, kc * TPC : (kc + 1) * TPC],
                        start=(hl == 0 and kc == 0),
                        stop=(last and kc == 7),
                        skip_group_check=True,
                    )
                if last:
                    yst = work_p.tile(
                        [128, TPC], f32, tag="yst", bufs=3, name=f"ys{b}{mt}"
                    )
                    nc.vector.tensor_scalar_add(yst[:], y_ap, bp_t[:, mt : mt + 1])
                    nc.sync.dma_start(
                        out=out_d[b, mt * 128 : (mt + 1) * 128, :], in_=yst[:]
                    )

            # ---- emission schedule ---------------------------------------
            # Phase 1: qkv(b0) back-to-back (dense PE stream ramps the
            # clock) + b0 v_aug transposes; b1 x-chunk DMAs pre-issued.
            emit_qkv_chunk(0, 0, xT_t=xT0)
            for rci in range(1, 4):
                emit_qkv_chunk(0, rci)
            for rci in range(4):
                load_xchunk(1, rci)
            nc.sync.dma_start(out=wp_t[:], in_=wp_d[:])
            emit_vaug(0, 0)
            emit_vaug(0, 1)

            # Phase 2: attention units.  qkv(b1), v_aug(b1), normalize
            # finishers, a2a/recv issues and proj blocks all run as filler
            # between kt iterations so the tensor engine never idles (idle
            # gaps drop the PE out of its boosted clock state).
            ai = {
                (b, hl): new_a2a_in(b, hl) for b in range(B) for hl in range(HPC)
            }
            ao = {}
            recv0 = get_recv(0)
            recv1 = get_recv(1)

            for rci in range(4):
                for m in range(3):
                    filler_q.append(
                        lambda rci=rci, m=m: emit_qkv_m(1, rci, m, act_ok=False)
                    )
            filler_q.append(lambda: emit_vaug(1, 0))
            filler_q.append(lambda: emit_vaug(1, 1))

            fin = emit_unit(0, 0, 0, ai[(0, 0)])
            filler_q.insert(0, fin)
            fin = emit_unit(0, 0, 1, ai[(0, 0)])
            filler_q.insert(0, fin)
            filler_q.insert(1, lambda: ao.update({(0, 0): emit_a2a(0, 0, ai[(0, 0)])}))
            fin = emit_unit(0, 1, 0, ai[(0, 1)])
            filler_q.insert(0, fin)
            fin = emit_unit(0, 1, 1, ai[(0, 1)])
            filler_q.insert(0, fin)
            filler_q.insert(1, lambda: ao.update({(0, 1): emit_a2a(0, 1, ai[(0, 1)])}))
            filler_q.insert(2, lambda: emit_recv_head(0, 0, ao[(0, 0)]))
            fin = emit_unit(1, 0, 0, ai[(1, 0)])
            filler_q.insert(0, fin)
            filler_q.insert(1, lambda: emit_recv_head(0, 1, ao[(0, 1)]))
            fin = emit_unit(1, 0, 1, ai[(1, 0)])
            filler_q.insert(0, fin)
            filler_q.insert(1, lambda: ao.update({(1, 0): emit_a2a(1, 0, ai[(1, 0)])}))
            for mt in range(4):
                filler_q.append(lambda mt=mt: emit_proj_mt(0, recv0, mt))
            fin = emit_unit(1, 1, 0, ai[(1, 1)])
            filler_q.insert(0, fin)
            filler_q.insert(1, lambda: emit_recv_head(1, 0, ao[(1, 0)]))
            for mt in range(4, 8):
                filler_q.append(lambda mt=mt: emit_proj_mt(0, recv0, mt))
            fin = emit_unit(1, 1, 1, ai[(1, 1)])
            while filler_q:
                pop_filler()
            fin()
            ao[(1, 1)] = emit_a2a(1, 1, ai[(1, 1)])

            # Phase 3: proj(1) in two K=64 passes -- pass A (head block 0)
            # overlaps the in-flight a2a(1,1); pass B after its receive.
            # PSUM matmul start=True resets the whole 2 KB bank, so every mt
            # accumulator must own a distinct bank: spread the 8 groups over
            # the (now idle) s/o/acc pool slots, one group per bank.
            y1a = ps.tile([128, 1024], f32, tag="s", bufs=2, name="y1a")
            y1b = ps.tile([128, 1024], f32, tag="s", bufs=2, name="y1b")
            y1c = ps.tile([128, 1024], f32, tag="o", bufs=1, name="y1c")
            y1d = ps.tile([128, 512], f32, tag="acc", bufs=2, name="y1d")
            y1e = ps.tile([128, 512], f32, tag="acc", bufs=2, name="y1e")

            def y_ap(mt):
                # mts 0-5: two per two-bank tile, one per bank (cols 0-255
                # in bank 0, cols 512-767 in bank 1); mts 6-7: one-bank tiles
                if mt < 6:
                    t = (y1a, y1b, y1c)[mt // 2]
                    return t[:, (mt % 2) * 512 : (mt % 2) * 512 + TPC]
                t = y1d if mt == 6 else y1e
                return t[:, 0:TPC]

            for mt in range(8):
                emit_proj_pass(1, recv1, mt, y_ap(mt), 0, last=False)
            emit_recv_head(1, 1, ao[(1, 1)])
            for mt in range(8):
                emit_proj_pass(1, recv1, mt, y_ap(mt), 1, last=True)

    _legalize_waits(nc)
    return nc


_NC_CACHE = None


def _get_nc():
    global _NC_CACHE
    if _NC_CACHE is None:
        _NC_CACHE = build_nc()
    return _NC_CACHE


def _make_in_maps(inputs):
    bf = ml_dtypes.bfloat16
    x = np.ascontiguousarray(np.asarray(inputs["x"], dtype=np.float32)).reshape(ROWS, C)
    xt = np.ascontiguousarray(x.T.astype(bf))   # [C, ROWS] bf16
    w_qkv = np.asarray(inputs["w_qkv"], dtype=np.float32)
    b_qkv = np.asarray(inputs["b_qkv"], dtype=np.float32)
    a_q = np.asarray(inputs["a_q"], dtype=np.float32)
    b_q = np.asarray(inputs["b_q"], dtype=np.float32)
    a_v = np.asarray(inputs["a_v"], dtype=np.float32)
    b_v = np.asarray(inputs["b_v"], dtype=np.float32)
    w_proj = np.asarray(inputs["w_proj"], dtype=np.float32)
    b_proj = np.asarray(inputs["b_proj"], dtype=np.float32)

    # fold the (linear) per-head LoRA into the q/v weights and biases:
    # q_final = (x@w_q + b_q) @ (I + a_q@b_q * scale)
    dq = a_q @ b_q * LORA_SCALE                 # [64, 64]
    dv = a_v @ b_v * LORA_SCALE
    mq = np.eye(128, dtype=np.float32)
    mq[0:64, 0:64] += dq
    mq[64:128, 64:128] += dq
    mv = np.eye(128, dtype=np.float32)
    mv[0:64, 0:64] += dv
    mv[64:128, 64:128] += dv

    eye64x2 = np.vstack([np.eye(64, dtype=np.float32)] * 2).astype(bf)

    def warr(w):                                # [1024, n] -> [128, 8*n] chunk-major
        n = w.shape[1]
        return np.ascontiguousarray(
            w.reshape(8, 128, n).transpose(1, 0, 2).reshape(128, 8 * n).astype(bf)
        )

    wp_full = warr(w_proj)                      # [128, 8*1024] bf16
    bp = np.ascontiguousarray(b_proj.reshape(8, 128).T)

    in_maps = []
    for c in range(NCORES):
        wq_c = w_qkv[:, 0 * C + c * PC : 0 * C + (c + 1) * PC] @ mq
        wk_c = w_qkv[:, 1 * C + c * PC : 1 * C + (c + 1) * PC]
        wv_c = w_qkv[:, 2 * C + c * PC : 2 * C + (c + 1) * PC] @ mv
        bq_c = b_qkv[0 * C + c * PC : 0 * C + (c + 1) * PC] @ mq
        bk_c = b_qkv[1 * C + c * PC : 1 * C + (c + 1) * PC]
        bv_c = b_qkv[2 * C + c * PC : 2 * C + (c + 1) * PC] @ mv
        in_maps.append(
            {
                "xt": xt,
                "wq": warr(wq_c),
                "wk": warr(wk_c),
                "wv": warr(wv_c),
                "bq": np.ascontiguousarray(bq_c.reshape(128, 1)),
                "bk": np.ascontiguousarray(bk_c.reshape(128, 1)),
                "bv": np.ascontiguousarray(bv_c.reshape(128, 1)),
                "wp": wp_full,
                "bp": bp,
                "eye64x2": eye64x2,
            }
        )
    return in_maps


def run_sharded(inputs, trace=False, **kw):
    nc = _get_nc()
    in_maps = _make_in_maps(inputs)
    res = run_bass_kernel_spmd(nc, in_maps, list(range(NCORES)), trace=trace, **kw)
    # results[c]["out"]: [B, C, TPC] -- core c's token shard of final y^T
    yT = np.concatenate([res.results[c]["out"] for c in range(NCORES)], axis=2)
    out = np.ascontiguousarray(yT.transpose(0, 2, 1))  # [B, N, C]
    return out, res


def kernel(**inputs) -> np.ndarray:
    out, _ = run_sharded(inputs, trace=False)
    return out


# revision 20
# speedup vs baseline: 1.3750x; 1.0390x over previous
"""Multi-head attention with q/v LoRA on 8 trn2 NeuronCores (bf16 PE path).

Reference computation (B=2, N=2048, C=1024, H=16, HD=64, R=16):
    qkv = x @ w_qkv + b_qkv                -> split per-head q, k, v
    q  += ((q @ a_q) @ b_q) * 2.0          (per head; same for v)
    out = softmax(q k^T / 8) v             (full N x N scores)
    y   = out @ w_proj + b_proj

Sharding: tensor-parallel over heads -- each of the 8 cores owns 2 heads
(128 of the 3*1024 qkv columns) for both batches; attention output is
resharded over tokens with a per-(batch,head) AllToAll so each core
computes final proj rows for its 256 tokens per batch with the full
w_proj.

Key implementation choices:
  * All PE operands are bf16 (hardware runs fp32r at ~2 cycles/row; bf16
    at 1).  PSUM accumulation stays fp32, biases stay fp32.
  * The LoRA is linear in q/v, so it is folded into the qkv weights on
    the host: w_eff = w @ (I + blockdiag(a@b)*scale), same for bias.
    Nothing LoRA-related runs on device.
  * x^T, weights are pre-cast to bf16 on the host and DMA'd straight
    into matmul operand tiles (no on-device rounding copies).
  * Softmax: scores S^T = k^T' q^T -> exp on ACT (bf16 out) -> P @ [v|1]
    in PSUM (ones column gives the row sums).  1/sums via the fast DVE
    reciprocal, broadcast to 64 partitions on the idle GpSimd engine,
    one fused multiply writes normalized bf16 O^T for the AllToAll.
  * v_aug ones columns are memset once into two persistent buffers.
  * AllToAll payloads are bf16 (256 KB per (batch, head)); the receive
    side DMAs the collective output straight into the proj operand tile.
The host stitches the 8 token shards and transposes back to [B, N, C].
"""

import sys

sys.path.insert(0, "/opt/trn_rl_repo")
sys.path.insert(0, "/root/.axon_site")

import numpy as np
import ml_dtypes

import concourse.bass as bass
import concourse.mybir as mybir
import concourse.tile as tile
from concourse.bass_utils import run_bass_kernel_spmd

f32 = mybir.dt.float32
bf16 = mybir.dt.bfloat16
AF = mybir.ActivationFunctionType

B, N, C = 2, 2048, 1024
H, HD, R = 16, 64, 16
LORA_SCALE = 32.0 / R
ATTN_SCALE = HD ** -0.5
NCORES = 8
HPC = H // NCORES          # heads per core = 2
PC = HPC * HD              # partition columns per core = 128
ROWS = B * N               # 4096 tokens
RC = 512                   # row-chunk size for qkv production
TPC = N // NCORES          # tokens per core per batch = 256


def _legalize_waits(nc, max_waits=1):
    """This walrus build accepts at most one sync-wait per instruction;
    Tile attaches several.  Move surplus waits onto same-engine NoOps
    inserted immediately before the instruction (identical semantics)."""
    counter = 0
    for fn in nc.m.functions:
        for bb in fn.blocks:
            insts = bb.instructions
            out = []
            changed = False
            for inst in insts:
                si = inst.sync_info
                if si is not None and si.on_wait and len(si.on_wait) > max_waits:
                    waits = list(si.on_wait)
                    for w in waits[:-max_waits]:
                        counter += 1
                        nop = mybir.InstNoOp(
                            name=f"I-wfix-{counter}",
                            engine=inst.engine,
                            sync_info=mybir.SyncInfo(on_wait=[w], on_update=[]),
                        )
                        nc.register_instruction(nop)
                        out.append(nop)
                    si.on_wait.clear()
                    si.on_wait.extend(waits[-max_waits:])
                    changed = True
                out.append(inst)
            if changed:
                insts[:] = out


def build_nc():
    nc = bass.Bass(num_devices=NCORES)

    xt_d = nc.dram_tensor("xt", [C, ROWS], bf16, kind="ExternalInput")
    wq_d = nc.dram_tensor("wq", [128, 1024], bf16, kind="ExternalInput")
    wk_d = nc.dram_tensor("wk", [128, 1024], bf16, kind="ExternalInput")
    wv_d = nc.dram_tensor("wv", [128, 1024], bf16, kind="ExternalInput")
    bq_d = nc.dram_tensor("bq", [128, 1], f32, kind="ExternalInput")
    bk_d = nc.dram_tensor("bk", [128, 1], f32, kind="ExternalInput")
    bv_d = nc.dram_tensor("bv", [128, 1], f32, kind="ExternalInput")
    wp_d = nc.dram_tensor("wp", [128, 8 * 1024], bf16, kind="ExternalInput")
    bp_d = nc.dram_tensor("bp", [128, 8], f32, kind="ExternalInput")
    eye64x2_d = nc.dram_tensor("eye64x2", [128, 64], bf16, kind="ExternalInput")
    out_d = nc.dram_tensor("out", [B, C, TPC], f32, kind="ExternalOutput")

    with nc.allow_low_precision(
        reason="bf16 matmul operands are intended; PSUM accumulation stays fp32"
    ), tile.TileContext(nc) as tc:
        with (
            tc.tile_pool(name="persist", bufs=1) as persist,
            tc.tile_pool(name="const", bufs=1) as const,
            tc.tile_pool(name="dram", bufs=1, space="DRAM") as dram,
            tc.tile_pool(name="xio", bufs=2) as xio_p,
            tc.tile_pool(name="work", bufs=2) as work_p,
            tc.tile_pool(name="ps", bufs=1, space="PSUM") as ps,
        ):
            qT = persist.tile([128, ROWS], bf16, tag="qT", name="qT")
            kT = persist.tile([128, ROWS], bf16, tag="kT", name="kT")
            vT = persist.tile([128, ROWS], bf16, tag="vT", name="vT")

            # prefetch the first x^T chunk ahead of the weight DMAs
            xT0 = xio_p.tile([128, 8 * RC], bf16, tag="xT", bufs=6, name="xT00")
            nc.sync.dma_start(
                out=xT0[:].rearrange("p (a r) -> p a r", a=8),
                in_=xt_d[:, 0:RC].rearrange("(a p) r -> p a r", p=128),
            )

            w_t = []
            for nm, d in (("wq", wq_d), ("wk", wk_d), ("wv", wv_d)):
                t = const.tile([128, 1024], bf16, tag=nm, name=f"{nm}_t")
                nc.sync.dma_start(out=t[:], in_=d[:])
                w_t.append(t)

            eye64x2 = const.tile([128, 64], bf16, tag="eye64", name="eye64")
            nc.sync.dma_start(out=eye64x2[:], in_=eye64x2_d[:])

            bias_t = []
            for nm, d in (("bq", bq_d), ("bk", bk_d), ("bv", bv_d)):
                bt = const.tile([128, 1], f32, tag=nm, name=f"{nm}_t")
                nc.sync.dma_start(out=bt[:], in_=d[:])
                bias_t.append(bt)
            bp_t = const.tile([128, 8], f32, tag="bp", name="bp_t")
            nc.sync.dma_start(out=bp_t[:], in_=bp_d[:])

            wp_t = const.tile([128, 8 * 1024], bf16, tag="wp_t", name="wp_t")

            # persistent v_aug buffers (one per (batch, head)): ones columns
            # written once by memset, data blocks overwritten by transposes
            v_aug_bufs = {}
            for b in range(B):
                for hl in range(HPC):
                    va = persist.tile(
                        [128, 16 * 65], bf16, tag=f"va{b}{hl}", name=f"va{b}{hl}"
                    )
                    nc.gpsimd.memset(va[:], 1.0)
                    v_aug_bufs[(b, hl)] = va

            ones_row = const.tile([1, 64], bf16, tag="ones_r", name="ones_r")
            nc.gpsimd.memset(ones_row[:], 1.0)

            qkvT = (qT, kT, vT)

            xts = {}

            def load_xchunk(b, rci):
                r0 = b * N + rci * RC
                xT_t = xio_p.tile(
                    [128, 8 * RC], bf16, tag="xT", bufs=6, name=f"xT{b}{rci}"
                )
                nc.sync.dma_start(
                    out=xT_t[:].rearrange("p (a r) -> p a r", a=8),
                    in_=xt_d[:, r0 : r0 + RC].rearrange("(a p) r -> p a r", p=128),
                )
                xts[(b, rci)] = xT_t
                return xT_t

            def emit_qkv_m(b, rci, m, act_ok=True):
                r0 = b * N + rci * RC
                xT_t = xts[(b, rci)]
                acc = ps.tile([128, RC], f32, tag="acc", bufs=2, name=f"ac{b}{rci}{m}")
                for ci in range(8):
                    nc.tensor.matmul(
                        acc[:],
                        w_t[m][:, ci * 128 : (ci + 1) * 128],
                        xT_t[:, ci * RC : (ci + 1) * RC],
                        start=(ci == 0),
                        stop=(ci == 7),
                    )
                dst = qkvT[m][:, r0 : r0 + RC]
                if m == 0 and act_ok:
                    nc.scalar.activation(dst, acc[:], AF.Identity, bias=bias_t[m][:])
                else:
                    nc.vector.tensor_scalar_add(dst, acc[:], bias_t[m][:])

            def emit_qkv_chunk(b, rci, xT_t=None, act_ok=True):
                if (b, rci) not in xts:
                    if xT_t is not None:
                        xts[(b, rci)] = xT_t
                    else:
                        load_xchunk(b, rci)
                for m in range(3):
                    emit_qkv_m(b, rci, m, act_ok=act_ok)

            def emit_vaug(b, hl):
                boff = b * N
                hs = slice(hl * HD, (hl + 1) * HD)
                v_aug = v_aug_bufs[(b, hl)]
                for kt4 in range(4):
                    vtr = ps.tile([128, 256], bf16, tag="s", bufs=2, name=f"vt{b}{hl}{kt4}")
                    for j in range(4):
                        ko = boff + (kt4 * 4 + j) * 128
                        nc.tensor.transpose(
                            vtr[:, j * 64 : (j + 1) * 64],
                            vT[hs, ko : ko + 128],
                            eye64x2[hs, :],
                        )
                    nc.vector.tensor_copy(
                        v_aug[:].rearrange("p (k e) -> p k e", e=65)[
                            :, kt4 * 4 : kt4 * 4 + 4, 0:64
                        ],
                        vtr[:].rearrange("p (k e) -> p k e", e=64),
                    )
                return v_aug

            # filler queue: small batches of independent PE work emitted
            # between attention kt iterations so the tensor engine never
            # starves while waiting on the ACT exp cadence
            filler_q = []

            def pop_filler():
                if filler_q:
                    filler_q.pop(0)()

            def emit_unit(b, hl, qh, a2a_in):
                """Emit scores/exp/PV for one (batch, head, q-half) unit.
                Returns a finisher closure (normalize + a2a staging DMA) to
                be emitted later -- after the next unit's first matmuls -- so
                the slow reciprocal never blocks the in-order PE queue."""
                boff = b * N
                hs = slice(hl * HD, (hl + 1) * HD)
                qoff = boff + qh * 1024
                v_aug = v_aug_bufs[(b, hl)]
                o_ps = ps.tile([65, 1024], f32, tag="o", bufs=1, name=f"o{b}{hl}{qh}")

                def emit_pv(p_tile, kt):
                    for qc in range(2):
                        nc.tensor.matmul(
                            o_ps[:, qc * 512 : (qc + 1) * 512],
                            v_aug[:, kt * 65 : kt * 65 + 65],
                            p_tile[:, qc * 512 : (qc + 1) * 512],
                            start=(kt == 0),
                            stop=(kt == 15),
                        )

                pending = None
                for kt in range(16):
                    ko = boff + kt * 128
                    s_ps = ps.tile([128, 1024], f32, tag="s", bufs=2, name=f"s{b}{hl}{qh}{kt}")
                    for qc in range(2):
                        nc.tensor.matmul(
                            s_ps[:, qc * 512 : (qc + 1) * 512],
                            kT[hs, ko : ko + 128],
                            qT[hs, qoff + qc * 512 : qoff + (qc + 1) * 512],
                            start=True,
                            stop=True,
                        )
                    p_sb = work_p.tile([128, 1024], bf16, tag="p", bufs=3, name=f"p{qh}{kt}")
                    nc.scalar.activation(p_sb[:], s_ps[:], AF.Exp, scale=ATTN_SCALE)
                    if pending is not None:
                        emit_pv(*pending)
                        if kt in (3, 7, 11):
                            pop_filler()
                    pending = (p_sb, kt)
                emit_pv(*pending)
                # DVE-side epilogue now (doesn't touch the PE queue): copy
                # O^T + sums out of PSUM, take reciprocals per q-half
                nst = work_p.tile([65, 1024], f32, tag="nst", bufs=2, name=f"n{hl}{qh}")
                nc.vector.tensor_copy(nst[:], o_ps[:])
                r_bf = work_p.tile([1, 1024], bf16, tag="rbf", bufs=2, name=f"rb{b}{hl}{qh}")
                for rq in range(4):
                    r_sb = work_p.tile([1, 256], f32, tag="r", bufs=2, name=f"r{b}{hl}{qh}{rq}")
                    nc.vector.reciprocal(r_sb[:], nst[64:65, rq * 256 : (rq + 1) * 256])
                    nc.vector.tensor_copy(r_bf[:, rq * 256 : (rq + 1) * 256], r_sb[:])

                def finisher():
                    onrm = work_p.tile(
                        [64, 1024], bf16, tag="onrm", bufs=2, name=f"on{b}{hl}{qh}"
                    )
                    for qc in range(2):
                        bc_ps = ps.tile([64, 512], f32, tag="acc", bufs=2, name=f"bc{qc}")
                        nc.tensor.matmul(
                            bc_ps[:],
                            ones_row[:],
                            r_bf[:, qc * 512 : (qc + 1) * 512],
                            start=True,
                            stop=True,
                        )
                        nc.vector.tensor_mul(
                            onrm[:, qc * 512 : (qc + 1) * 512],
                            nst[0:64, qc * 512 : (qc + 1) * 512],
                            bc_ps[:],
                        )
                    nc.sync.dma_start(
                        out=a2a_in[qh * 4 : (qh + 1) * 4, :, :].rearrange(
                            "a p r -> p a r"
                        ),
                        in_=onrm[:].rearrange("p (a r) -> p a r", a=4),
                    )

                return finisher

            def emit_a2a(b, hl, a2a_in):
                a2a_out = dram.tile(
                    [8, 64, TPC], bf16, tag=f"ao{b}{hl}", name=f"ao{b}{hl}"
                )
                nc.gpsimd.collective_compute(
                    "AllToAll",
                    mybir.AluOpType.bypass,
                    replica_groups=[list(range(NCORES))],
                    ins=[a2a_in[:].opt()],
                    outs=[a2a_out[:].opt()],
                )
                return a2a_out

            def new_a2a_in(b, hl):
                return dram.tile([8, 64, TPC], bf16, tag=f"ai{b}{hl}", name=f"ai{b}{hl}")

            recv_tiles = {}

            def get_recv(b):
                if b not in recv_tiles:
                    recv_tiles[b] = work_p.tile(
                        [128, 8 * TPC], bf16, tag=f"rcr{b}", bufs=1, name=f"rr{b}"
                    )
                return recv_tiles[b]

            def emit_recv_head(b, hl, a2a_out):
                recv_r = get_recv(b)
                nc.sync.dma_start(
                    out=recv_r[hl * 64 : (hl + 1) * 64, :].rearrange(
                        "p (a r) -> p a r", a=8
                    ),
                    in_=a2a_out[:].rearrange("a p r -> p a r"),
                )
                return recv_r

            def emit_proj_mt(b, recv_r, mt):
                y_ps = ps.tile([128, TPC], f32, tag="acc", bufs=2, name=f"y{b}{mt}")
                for kc in range(8):
                    nc.tensor.matmul(
                        y_ps[:],
                        wp_t[:, kc * 1024 + mt * 128 : kc * 1024 + (mt + 1) * 128],
                        recv_r[:, kc * TPC : (kc + 1) * TPC],
                        start=(kc == 0),
                        stop=(kc == 7),
                    )
                yst = work_p.tile([128, TPC], f32, tag="yst", bufs=3, name=f"ys{b}{mt}")
                nc.vector.tensor_scalar_add(yst[:], y_ps[:], bp_t[:, mt : mt + 1])
                nc.sync.dma_start(
                    out=out_d[b, mt * 128 : (mt + 1) * 128, :], in_=yst[:]
                )

            def emit_proj_pass(b, recv_r, mt, y_ap, hl, last):
                """K=64 proj pass over one head-pair block; accumulates into
                y_ap across the two passes (hl=0 start, hl=1 stop)."""
                lo, hi = hl * 64, (hl + 1) * 64
                for kc in range(8):
                    nc.tensor.matmul(
                        y_ap,
                        wp_t[lo:hi, kc * 1024 + mt * 128 : kc * 1024 + (mt + 1) * 128],
                        recv_r[lo:hi, kc * TPC : (kc + 1) * TPC],
                        start=(hl == 0 and kc == 0),
                        stop=(last and kc == 7),
                        skip_group_check=True,
                    )
                if last:
                    yst = work_p.tile(
                        [128, TPC], f32, tag="yst", bufs=3, name=f"ys{b}{mt}"
                    )
                    nc.vector.tensor_scalar_add(yst[:], y_ap, bp_t[:, mt : mt + 1])
                    nc.sync.dma_start(
                        out=out_d[b, mt * 128 : (mt + 1) * 128, :], in_=yst[:]
                    )

            # ---- emission schedule ---------------------------------------
            # Phase 1: qkv(b0) back-to-back (dense PE stream ramps the
            # clock) + b0 v_aug transposes; b1 x-chunk DMAs pre-issued.
            emit_qkv_chunk(0, 0, xT_t=xT0)
            for rci in range(1, 4):
                emit_qkv_chunk(0, rci)
            for rci in range(4):
                load_xchunk(1, rci)
            nc.sync.dma_start(out=wp_t[:], in_=wp_d[:])
            emit_vaug(0, 0)
            emit_vaug(0, 1)

            # Phase 2: attention units.  qkv(b1), v_aug(b1), normalize
            # finishers, a2a/recv issues and proj blocks all run as filler
            # between kt iterations so the tensor engine never idles (idle
            # gaps drop the PE out of its boosted clock state).
            ai = {
                (b, hl): new_a2a_in(b, hl) for b in range(B) for hl in range(HPC)
            }
            ao = {}
            recv0 = get_recv(0)
            recv1 = get_recv(1)

            for rci in range(4):
                for m in range(3):
                    filler_q.append(
                        lambda rci=rci, m=m: emit_qkv_m(1, rci, m, act_ok=False)
                    )
            filler_q.append(lambda: emit_vaug(1, 0))
            filler_q.append(lambda: emit_vaug(1, 1))

            fin = emit_unit(0, 0, 0, ai[(0, 0)])
            filler_q.insert(0, fin)
            fin = emit_unit(0, 0, 1, ai[(0, 0)])
            filler_q.insert(0, fin)
            filler_q.insert(1, lambda: ao.update({(0, 0): emit_a2a(0, 0, ai[(0, 0)])}))
            fin = emit_unit(0, 1, 0, ai[(0, 1)])
            filler_q.insert(0, fin)
            fin = emit_unit(0, 1, 1, ai[(0, 1)])
            filler_q.insert(0, fin)
            filler_q.insert(1, lambda: ao.update({(0, 1): emit_a2a(0, 1, ai[(0, 1)])}))
            filler_q.insert(2, lambda: emit_recv_head(0, 0, ao[(0, 0)]))
            # b1 attention consumes qkv(b1)/v_aug(b1): drain any fillers
            # that haven't been popped yet before the first b1 unit
            while filler_q:
                pop_filler()
            fin = emit_unit(1, 0, 0, ai[(1, 0)])
            filler_q.insert(0, fin)
            filler_q.insert(1, lambda: emit_recv_head(0, 1, ao[(0, 1)]))
            fin = emit_unit(1, 0, 1, ai[(1, 0)])
            filler_q.insert(0, fin)
            filler_q.insert(1, lambda: ao.update({(1, 0): emit_a2a(1, 0, ai[(1, 0)])}))
            for mt in range(4):
                filler_q.append(lambda mt=mt: emit_proj_mt(0, recv0, mt))
            fin = emit_unit(1, 1, 0, ai[(1, 1)])
            filler_q.insert(0, fin)
            filler_q.insert(1, lambda: emit_recv_head(1, 0, ao[(1, 0)]))
            for mt in range(4, 8):
                filler_q.append(lambda mt=mt: emit_proj_mt(0, recv0, mt))
            fin = emit_unit(1, 1, 1, ai[(1, 1)])
            while filler_q:
                pop_filler()
            fin()
            ao[(1, 1)] = emit_a2a(1, 1, ai[(1, 1)])

            # Phase 3: proj(1) in two K=64 passes -- pass A (head block 0)
            # overlaps the in-flight a2a(1,1); pass B after its receive.
            # PSUM matmul start=True resets the whole 2 KB bank, so every mt
            # accumulator must own a distinct bank: spread the 8 groups over
            # the (now idle) s/o/acc pool slots, one group per bank.
            y1a = ps.tile([128, 1024], f32, tag="s", bufs=2, name="y1a")
            y1b = ps.tile([128, 1024], f32, tag="s", bufs=2, name="y1b")
            y1c = ps.tile([128, 1024], f32, tag="o", bufs=1, name="y1c")
            y1d = ps.tile([128, 512], f32, tag="acc", bufs=2, name="y1d")
            y1e = ps.tile([128, 512], f32, tag="acc", bufs=2, name="y1e")

            def y_ap(mt):
                # mts 0-5: two per two-bank tile, one per bank (cols 0-255
                # in bank 0, cols 512-767 in bank 1); mts 6-7: one-bank tiles
                if mt < 6:
                    t = (y1a, y1b, y1c)[mt // 2]
                    return t[:, (mt % 2) * 512 : (mt % 2) * 512 + TPC]
                t = y1d if mt == 6 else y1e
                return t[:, 0:TPC]

            for mt in range(8):
                emit_proj_pass(1, recv1, mt, y_ap(mt), 0, last=False)
            emit_recv_head(1, 1, ao[(1, 1)])
            for mt in range(8):
                emit_proj_pass(1, recv1, mt, y_ap(mt), 1, last=True)

    _legalize_waits(nc)
    return nc


_NC_CACHE = None


def _get_nc():
    global _NC_CACHE
    if _NC_CACHE is None:
        _NC_CACHE = build_nc()
    return _NC_CACHE


def _make_in_maps(inputs):
    bf = ml_dtypes.bfloat16
    x = np.ascontiguousarray(np.asarray(inputs["x"], dtype=np.float32)).reshape(ROWS, C)
    xt = np.ascontiguousarray(x.T.astype(bf))   # [C, ROWS] bf16
    w_qkv = np.asarray(inputs["w_qkv"], dtype=np.float32)
    b_qkv = np.asarray(inputs["b_qkv"], dtype=np.float32)
    a_q = np.asarray(inputs["a_q"], dtype=np.float32)
    b_q = np.asarray(inputs["b_q"], dtype=np.float32)
    a_v = np.asarray(inputs["a_v"], dtype=np.float32)
    b_v = np.asarray(inputs["b_v"], dtype=np.float32)
    w_proj = np.asarray(inputs["w_proj"], dtype=np.float32)
    b_proj = np.asarray(inputs["b_proj"], dtype=np.float32)

    # fold the (linear) per-head LoRA into the q/v weights and biases:
    # q_final = (x@w_q + b_q) @ (I + a_q@b_q * scale)
    dq = a_q @ b_q * LORA_SCALE                 # [64, 64]
    dv = a_v @ b_v * LORA_SCALE
    mq = np.eye(128, dtype=np.float32)
    mq[0:64, 0:64] += dq
    mq[64:128, 64:128] += dq
    mv = np.eye(128, dtype=np.float32)
    mv[0:64, 0:64] += dv
    mv[64:128, 64:128] += dv

    eye64x2 = np.vstack([np.eye(64, dtype=np.float32)] * 2).astype(bf)

    def warr(w):                                # [1024, n] -> [128, 8*n] chunk-major
        n = w.shape[1]
        return np.ascontiguousarray(
            w.reshape(8, 128, n).transpose(1, 0, 2).reshape(128, 8 * n).astype(bf)
        )

    wp_full = warr(w_proj)                      # [128, 8*1024] bf16
    bp = np.ascontiguousarray(b_proj.reshape(8, 128).T)

    in_maps = []
    for c in range(NCORES):
        wq_c = w_qkv[:, 0 * C + c * PC : 0 * C + (c + 1) * PC] @ mq
        wk_c = w_qkv[:, 1 * C + c * PC : 1 * C + (c + 1) * PC]
        wv_c = w_qkv[:, 2 * C + c * PC : 2 * C + (c + 1) * PC] @ mv
        bq_c = b_qkv[0 * C + c * PC : 0 * C + (c + 1) * PC] @ mq
        bk_c = b_qkv[1 * C + c * PC : 1 * C + (c + 1) * PC]
        bv_c = b_qkv[2 * C + c * PC : 2 * C + (c + 1) * PC] @ mv
        in_maps.append(
            {
                "xt": xt,
                "wq": warr(wq_c),
                "wk": warr(wk_c),
                "wv": warr(wv_c),
                "bq": np.ascontiguousarray(bq_c.reshape(128, 1)),
                "bk": np.ascontiguousarray(bk_c.reshape(128, 1)),
                "bv": np.ascontiguousarray(bv_c.reshape(128, 1)),
                "wp": wp_full,
                "bp": bp,
                "eye64x2": eye64x2,
            }
        )
    return in_maps


def run_sharded(inputs, trace=False, **kw):
    nc = _get_nc()
    in_maps = _make_in_maps(inputs)
    res = run_bass_kernel_spmd(nc, in_maps, list(range(NCORES)), trace=trace, **kw)
    # results[c]["out"]: [B, C, TPC] -- core c's token shard of final y^T
    yT = np.concatenate([res.results[c]["out"] for c in range(NCORES)], axis=2)
    out = np.ascontiguousarray(yT.transpose(0, 2, 1))  # [B, N, C]
    return out, res


def kernel(**inputs) -> np.ndarray:
    out, _ = run_sharded(inputs, trace=False)
    return out
